# revision 1
# baseline (speedup 1.0000x reference)
"""Trainium2 Bass kernel for nn_Attention_54391465836966.

Math (per batch b):
  ctok = content_feat[b].reshape(S,C) + pos            # [1024, 512]
  comp_tok[n] = components[n,b].reshape(S,C) + pos
  q = ctok @ Wq ; k[n],v[n] = comp_tok[n] @ Wkv (split)
  per head h, comp n: P = exp(scale * q_h k_h^T); o_nh = (P @ v_nh) / rowsum(P)
  result = sum_n o_n ; s = (result + ctok) @ Wproj + bproj
  out = Wconv[:, :512] @ s2d + Wconv[:, 512:] @ cf2d + bconv
    where s2d = s buffer reinterpreted [512, 1024], cf2d = content_feat[b] as [512, 1024]

Sharding: 8 cores <- (b, n) pairs; b = core//4, n = core%4.  Everything after
`result` is affine in the component partial, so each core applies the linear
tail to its own o_n (the constant terms -- ctok path, biases, cf2d conv -- are
gated to the n==0 core via zeroed per-core inputs) and the host sums the four
partial outputs per batch.  No collectives.

All matmuls run as float32r (~1e-4 relerr, full PE rate).  Scores are computed
transposed (S^T[ki,q]) so softmax-sum lands on the matmul contraction via an
augmented ones-column in V; normalization uses exp(-ln Z) on the ACT engine
(both fns in one table set).
"""
import sys

sys.path.insert(0, "/opt/trn_rl_repo")

import numpy as np

N_CORES = 8
B, C, H, W = 2, 512, 32, 32
S = H * W  # 1024
NH, HD = 8, 64
SCALE = HD ** -0.5

_CACHE = {}


def _build():
    if "nc" in _CACHE:
        return _CACHE["nc"]
    from contextlib import ExitStack

    import concourse.bacc as bacc
    import concourse.mybir as mybir
    import concourse.tile as tile
    from concourse.masks import make_identity

    f32 = mybir.dt.float32
    f32r = mybir.dt.float32r
    EXP = mybir.ActivationFunctionType.Exp

    nc = bacc.Bacc("TRN2", target_bir_lowering=False, debug=False,
                   num_devices=N_CORES)

    # weights / biases declared float32r so they can feed fp32r matmuls
    # straight from DMA (same bits as f32 host-side)
    din = lambda n, s, dt: nc.dram_tensor(n, s, dt, kind="ExternalInput").ap()
    cf = din("cf", [C, S], f32)        # content_feat[b], c-major
    comp = din("comp", [C, S], f32)    # components[n,b], c-major
    pos = din("pos", [S, C], f32)
    wq = din("wq", [C, C], f32r)
    wkv = din("wkv", [C, 2 * C], f32r)
    wproj = din("wproj", [C, C], f32r)
    wconv = din("wconv", [C, 2 * C], f32r)  # cols 512: zeroed for n>0 cores
    bproj = din("bproj", [1, C], f32r)      # zeroed for n>0 cores
    bconv = din("bconv", [1, C], f32r)      # zeroed for n>0 cores
    gate = din("gate", [128, 1], f32)       # 1.0 on n==0 cores else 0.0
    out_p = nc.dram_tensor("out_p", [C, S], f32, kind="ExternalOutput").ap()

    cf_tok = cf.rearrange("a (b c) -> (a b) c", b=2)      # [1024, 512] token view
    comp_tok = comp.rearrange("a (b c) -> (a b) c", b=2)  # [1024, 512]
    cf2d = cf.bitcast(f32r)                                # [512, 1024] c-major

    with tile.TileContext(nc) as tc, ExitStack() as ctx:
        main = ctx.enter_context(tc.tile_pool(name="main", bufs=1))
        trans = ctx.enter_context(tc.tile_pool(name="trans", bufs=2))
        dramp = ctx.enter_context(tc.tile_pool(name="dramp", bufs=1, space="DRAM"))

        # ---- constants ----
        ident = main.tile([128, 128], f32r, tag="ident", name="ident_v17")
        ident32 = trans.tile([128, 128], f32, tag="cn", bufs=4)
        make_identity(nc, ident32[:])
        nc.vector.tensor_copy(ident[:], ident32[:])
        ones = main.tile([128, 512], f32r, tag="ones")
        ones32 = trans.tile([128, 512], f32, tag="cnr", bufs=3)
        nc.gpsimd.memset(ones32[:], 1.0)
        nc.vector.tensor_copy(ones[:], ones32[:])
        g_sb = main.tile([128, 1], f32, tag="g")
        bproj_r = main.tile([1, C], f32r, tag="bpr")
        bconv_r = main.tile([1, C], f32r, tag="bcr")

        # one PSUM pool spans setup + attention so the scheduler can overlap
        # them: mm(1 bank x2) + sc(2 banks x2) + o(2 banks x1) = 8 banks
        with tc.tile_pool(name="psAB", bufs=2, space="PSUM") as ps:
            # ---- token transposes ----
            ctokT = [main.tile([128, S], f32r, tag=f"ctokT{j}", name=f"ctokT{j}")
                     for j in range(4)]
            compT = [main.tile([128, S], f32r, tag=f"cr{j}", name=f"compT{j}",
                               bufs=2) for j in range(4)]
            for t in range(8):
                pos_t = trans.tile([128, C], f32, tag="pos", bufs=3)
                nc.sync.dma_start(pos_t[:], pos[128 * t:128 * (t + 1), :])
                for src, dstT, nm in ((cf_tok, ctokT, "cna"), (comp_tok, compT, "cnb")):
                    nat = trans.tile([128, C], f32, tag="cn", name=nm, bufs=4)
                    nc.sync.dma_start(nat[:], src[128 * t:128 * (t + 1), :])
                    natr = trans.tile([128, C], f32r, tag="cnr", name=nm + "r", bufs=3)
                    # split the pos-adds across DVE and the idle Pool engine
                    if nm == "cna":
                        nc.vector.tensor_add(natr[:], nat[:], pos_t[:])
                    else:
                        nc.gpsimd.tensor_add(natr[:], nat[:], pos_t[:])
                    for j in range(4):
                        tp = ps.tile([128, 128], f32r, tag="mm")
                        nc.tensor.transpose(tp[:], natr[:, 128 * j:128 * (j + 1)],
                                            ident[:])
                        if nm == "cna":
                            nc.vector.tensor_copy(
                                dstT[j][:, 128 * t:128 * (t + 1)], tp[:])
                        else:
                            nc.scalar.copy(dstT[j][:, 128 * t:128 * (t + 1)], tp[:])

            # ---- weights ----
            wq_r = [main.tile([128, C], f32r, tag=f"wq{k}", name=f"wq{k}")
                    for k in range(4)]
            wkv_r = [main.tile([128, 2 * C], f32r, tag=f"wkv{k}", name=f"wkv{k}")
                     for k in range(4)]
            for k in range(4):
                nc.sync.dma_start(wkv_r[k][:], wkv[128 * k:128 * (k + 1), :])
            for k in range(4):
                nc.sync.dma_start(wq_r[k][:], wq[128 * k:128 * (k + 1), :])
            wconvT = [main.tile([128, C], f32r, tag=f"wcT{j}", name=f"wcT{j}")
                      for j in range(8)]
            # late-needed consts: emitted after the token stream so they
            # don't delay the first transposes in the DMA queue
            nc.sync.dma_start(g_sb[:], gate[:])
            nc.sync.dma_start(bproj_r[:], bproj[:])
            nc.sync.dma_start(bconv_r[:], bconv[:])
            # odd heads need Wproj rows at base partition 0 (matmul base
            # rule); even heads read slices of the wproj4 tiles.  The odd
            # tiles ride transient-pool tags that die after startup.
            wproj_odd = []
            for p, (tg, bf) in enumerate((("cn", 4), ("cnr", 3), ("pos", 3))):
                w = trans.tile([64, C], f32r, tag=tg, name=f"wpo{p}", bufs=bf)
                nc.sync.dma_start(w[:], wproj[64 * (2 * p + 1):64 * (2 * p + 2), :])
                wproj_odd.append(w)

            # ---- v (first: gates attention start), then kT/qT ----
            v_sb = [main.tile([128, 8 * 65], f32r, tag=f"v{t}", name=f"v{t}")
                    for t in range(8)]
            for t in range(8):
                nc.scalar.copy(
                    v_sb[t][:].rearrange("p (h e) -> p h e", h=8)[:, :, 64:65],
                    ones[:, 0:8].rearrange("p (h o) -> p h o", o=1))
                acc = ps.tile([128, 512], f32, tag="mm")
                for k in range(4):
                    nc.tensor.matmul(acc[:], compT[k][:, 128 * t:128 * (t + 1)],
                                     wkv_r[k][:, C:2 * C],
                                     start=(k == 0), stop=(k == 3))
                nc.scalar.copy(
                    v_sb[t][:].rearrange("p (h e) -> p h e", h=8)[:, :, 0:64],
                    acc[:].rearrange("p (h d) -> p h d", h=8))

            qT = [main.tile([128, S], f32r, tag=f"qT{j}", name=f"qT{j}")
                  for j in range(4)]
            kT = [main.tile([128, S], f32r, tag=f"kT{j}", name=f"kT{j}")
                  for j in range(4)]
            for j in range(4):
                for tck in range(2):
                    for dst, wsrc, act in ((kT, wkv_r, compT), (qT, wq_r, ctokT)):
                        acc = ps.tile([128, 512], f32, tag="mm")
                        for k in range(4):
                            nc.tensor.matmul(acc[:],
                                             wsrc[k][:, 128 * j:128 * (j + 1)],
                                             act[k][:, 512 * tck:512 * (tck + 1)],
                                             start=(k == 0), stop=(k == 3))
                        nc.vector.tensor_copy(
                            dst[j][:, 512 * tck:512 * (tck + 1)], acc[:])

            # Wconv transpose (cheap PE; copies on DVE)
            for i in range(4):
                wcr = trans.tile([128, 2 * C], f32r, tag="wcr", bufs=1)
                nc.sync.dma_start(wcr[:], wconv[128 * i:128 * (i + 1), :])
                for j in range(8):
                    tp = ps.tile([128, 128], f32r, tag="mm")
                    nc.tensor.transpose(tp[:], wcr[:, 128 * j:128 * (j + 1)],
                                        ident[:])
                    nc.vector.tensor_copy(wconvT[j][:, 128 * i:128 * (i + 1)],
                                          tp[:])

            # ---- attention + interleaved per-head normalization ----
            rTu = [main.tile([65, S], f32r, tag=f"cr{h // 2}", name=f"rTu{h}",
                             bufs=2) for h in range(NH)]
            zscr = main.tile([65, S], f32, tag="zscr")
            zinv = main.tile([65, S], f32, tag="zinv")
            zs2 = main.tile([1, S], f32, tag="zs2")
            for h in range(NH):
                jq, row = h // 2, 64 * (h % 2)
                o_ps = ps.tile([65, S], f32, tag="o", bufs=1)
                for kt in range(8):
                    sc = ps.tile([128, S], f32, tag="sc", bufs=2)
                    for qc in range(2):
                        nc.tensor.matmul(
                            sc[:, 512 * qc:512 * (qc + 1)],
                            kT[jq][row:row + 64, 128 * kt:128 * (kt + 1)],
                            qT[jq][row:row + 64, 512 * qc:512 * (qc + 1)],
                            start=True, stop=True)
                    pt = main.tile([128, S], f32r, tag=f"wq{kt % 4}",
                                   name=f"pt{kt}")
                    nc.scalar.activation(pt[:], sc[:], EXP, scale=SCALE)
                    for qc in range(2):
                        nc.tensor.matmul(
                            o_ps[:, 512 * qc:512 * (qc + 1)],
                            v_sb[kt][:, 65 * h:65 * h + 65],
                            pt[:, 512 * qc:512 * (qc + 1)],
                            start=(kt == 0), stop=(kt == 7))
                # custom-DVE recip and partition_broadcast both need base
                # partition 0 on HW: shift the Z row down first (1-input
                # copies may change base partition).  For the last head the
                # recip chain starts straight from PSUM on DVE while the rTu
                # copy runs on the idle ACT engine, shortening the tail gate.
                if h == NH - 1:
                    nc.vector.tensor_copy(zscr[0:1, :], o_ps[64:65, :])
                    nc.scalar.copy(rTu[h][:], o_ps[:])
                else:
                    nc.vector.tensor_copy(rTu[h][:], o_ps[:])
                    nc.vector.tensor_copy(zscr[0:1, :], rTu[h][64:65, :])
                nc.vector.reciprocal_approx_accurate(
                    zinv[0:1, :], zscr[0:1, :], zs2[0:1, :])
                nc.gpsimd.partition_broadcast(zscr[0:64, :], zinv[0:1, :])
                mul_eng = nc.vector if h == NH - 1 else nc.gpsimd
                mul_eng.tensor_mul(rTu[h][0:64, :], rTu[h][0:64, :],
                                   zscr[0:64, :])

            wpo3 = trans.tile([64, C], f32r, tag="wcr", name="wpo3", bufs=1)
            nc.sync.dma_start(wpo3[:], wproj[64 * 7:64 * 8, :])
            wproj_odd.append(wpo3)

            # ---- cf2d partial conv: emitted after attention so it fills the
            # ACT-bound PE gaps; rides the dead wkv tags ----
            cf2d_r = [main.tile([128, S], f32r, tag=f"wkv{j}", name=f"c2r{j}")
                      for j in range(4)]
            for j in range(4):
                nc.sync.dma_start(cf2d_r[j][:], cf2d[128 * j:128 * (j + 1), :])
            outpart = [main.tile([128, S], f32, tag=f"op{oc}", name=f"op{oc}")
                       for oc in range(4)]
            for oc in range(4):
                for pc in range(2):
                    acc = ps.tile([128, 512], f32, tag="mm")
                    nc.tensor.matmul(acc[:], bconv_r[0:1, 128 * oc:128 * (oc + 1)],
                                     ones[0:1, :], start=True, stop=False)
                    for k2 in range(4):
                        nc.tensor.matmul(acc[:],
                                         wconvT[4 + k2][:, 128 * oc:128 * (oc + 1)],
                                         cf2d_r[k2][:, 512 * pc:512 * (pc + 1)],
                                         start=False, stop=(k2 == 3))
                    nc.vector.tensor_copy(outpart[oc][:, 512 * pc:512 * (pc + 1)],
                                          acc[:])

        # gate ctokT in place (only read by the proj matmuls afterwards)
        for j in range(4):
            nc.vector.tensor_scalar_mul(ctokT[j][:], ctokT[j][:], g_sb[:, 0:1])
        # second copy of Wproj in 4x[128,C] layout for the gated-ctok proj
        # terms; rides the wkv tags after cf2d
        wproj4 = [main.tile([128, C], f32r, tag=f"wkv{j}", name=f"wp4_{j}")
                  for j in range(4)]
        for j in range(4):
            nc.sync.dma_start(wproj4[j][:], wproj[128 * j:128 * (j + 1), :])

        # ---- proj + conv tail (pipelined through DRAM in 4 chunks) ----
        st_dram = [dramp.tile([128, C], f32r, name=f"stt{t}") for t in range(8)]
        with tc.tile_pool(name="psC", bufs=2, space="PSUM") as psC:
            for t in range(8):
                acc = psC.tile([128, 512], f32, tag="mm2")
                nc.tensor.matmul(acc[:], ones[0:1, 0:128], bproj_r[:],
                                 start=True, stop=False)
                for j in range(4):
                    nc.tensor.matmul(acc[:],
                                     ctokT[j][:, 128 * t:128 * (t + 1)],
                                     wproj4[j][:], start=False, stop=False)
                for h in range(NH):
                    wp_rhs = (wproj4[h // 2][0:64, :] if h % 2 == 0
                              else wproj_odd[h // 2][:])
                    nc.tensor.matmul(acc[:],
                                     rTu[h][0:64, 128 * t:128 * (t + 1)],
                                     wp_rhs,
                                     start=False, stop=(h == NH - 1))
                st = main.tile([128, C], f32r, tag=f"st{t % 2}", name=f"st{t}")
                nc.scalar.copy(st[:], acc[:])
                nc.sync.dma_start(st_dram[t][:, :], st[:])

            # each half of an s2d chunk depends on only one proj tile's store,
            # so the reload pipelines per-tile instead of per-chunk
            s2d_sb = []
            for j in range(4):
                sj = main.tile([128, S], f32r, tag=f"qT{j}", name=f"s2d{j}")
                for half in range(2):
                    hv = st_dram[2 * j + half][:].rearrange(
                        "(a b) c -> a (b c)", a=64, b=2)
                    nc.sync.dma_start(sj[64 * half:64 * half + 64, :], hv[:, :])
                s2d_sb.append(sj)
            for oc in range(4):
                for pc in range(2):
                    acc = psC.tile([128, 512], f32, tag="cv", bufs=6)
                    for j in range(4):
                        nc.tensor.matmul(acc[:],
                                         wconvT[j][:, 128 * oc:128 * (oc + 1)],
                                         s2d_sb[j][:, 512 * pc:512 * (pc + 1)],
                                         start=(j == 0), stop=(j == 3))
                    nc.vector.tensor_add(
                        outpart[oc][:, 512 * pc:512 * (pc + 1)],
                        outpart[oc][:, 512 * pc:512 * (pc + 1)], acc[:])
                    nc.sync.dma_start(
                        out_p[128 * oc:128 * (oc + 1),
                              512 * pc:512 * (pc + 1)],
                        outpart[oc][:, 512 * pc:512 * (pc + 1)])

    nc.compile()
    _CACHE["nc"] = nc
    return nc


def _shard_inputs(content_feat, components, pos_emb, Wq, Wkv, Wproj, bproj,
                  Wconv, bconv):
    f = np.float32
    pos2 = np.ascontiguousarray(pos_emb.reshape(S, C), dtype=f)
    wq2 = np.ascontiguousarray(Wq, dtype=f)
    wkv2 = np.ascontiguousarray(Wkv, dtype=f)
    wp2 = np.ascontiguousarray(Wproj, dtype=f)
    wc_first = np.ascontiguousarray(Wconv, dtype=f)
    wc_rest = wc_first.copy()
    wc_rest[:, C:] = 0.0
    bp1 = np.ascontiguousarray(bproj.reshape(1, C), dtype=f)
    bc1 = np.ascontiguousarray(bconv.reshape(1, C), dtype=f)
    zeros1 = np.zeros((1, C), dtype=f)
    in_maps = []
    for core in range(N_CORES):
        b, n = core // 4, core % 4
        first = n == 0
        in_maps.append({
            "cf": np.ascontiguousarray(content_feat[b].reshape(C, S), dtype=f),
            "comp": np.ascontiguousarray(components[n, b].reshape(C, S), dtype=f),
            "pos": pos2,
            "wq": wq2,
            "wkv": wkv2,
            "wproj": wp2,
            "wconv": wc_first if first else wc_rest,
            "bproj": bp1 if first else zeros1,
            "bconv": bc1 if first else zeros1,
            "gate": np.full((128, 1), 1.0 if first else 0.0, dtype=f),
        })
    return in_maps


def _run(trace=False, **inputs):
    from concourse.bass_utils import run_bass_kernel_spmd

    nc = _build()
    in_maps = _shard_inputs(**inputs)
    res = run_bass_kernel_spmd(nc, in_maps, list(range(N_CORES)), trace=trace)
    outs = [res.results[i]["out_p"] for i in range(N_CORES)]
    out = np.stack([outs[0] + outs[1] + outs[2] + outs[3],
                    outs[4] + outs[5] + outs[6] + outs[7]], axis=0)
    return out.reshape(B, C, H, W).astype(np.float32), res


def kernel(**inputs):
    out, _ = _run(trace=False, **inputs)
    return out



# revision 8
# speedup vs baseline: 1.0865x; 1.0865x over previous
"""Trainium2 Bass kernel for nn_Attention_54391465836966.

Math (per batch b, component n; one (b, n) pair per core):
  ctok = content_feat[b].reshape(S,C) + pos           # raw reshape tokens
  comp_tok = components[n,b].reshape(S,C) + pos
  q = ctok @ Wq ; k,v = comp_tok @ Wkv (split)
  per head h: P' = exp(scale*q k^T - 12); o_h = (P' @ v) / rowsum(P')
  result = sum_n o_n ; s = (result + ctok) @ Wproj + bproj
  out = Wconv[:, :512] . s2d + Wconv[:, 512:] . cf2d + bconv
    (s2d = raw [C, S] view of the token-major s buffer)

All device data is fp16 (f32 PSUM accumulation); exp carries a constant
-12 bias so probabilities fit fp16 (softmax is invariant to it).  Host
passes token-channel-major transposes (comp^T, cf^T, pos^T) and Wconv^T,
so the kernel does zero PE transposes.  Query tokens are parity-permuted
(even tokens first) end-to-end: the proj output tiles then directly ARE
the raw-reshape s2d chunks the conv needs, removing the DRAM round-trip.
The (result + ctok) constant term rides the attention output as
+0.25*ctok^T per core (host sums 4 component partials per batch), and
bproj/bconv are quartered host-side, so no per-core gating is needed
except zeroing the cf-half of Wconv^T on n>0 cores.
"""
import sys

sys.path.insert(0, "/opt/trn_rl_repo")

import numpy as np

N_CORES = 8
B, C, H, W = 2, 512, 32, 32
S = H * W  # 1024
NH, HD = 8, 64
SCALE = HD ** -0.5
EXP_BIAS = -12.0

_CACHE = {}


def _build():
    if "nc" in _CACHE:
        return _CACHE["nc"]
    from contextlib import ExitStack

    import concourse.bacc as bacc
    import concourse.mybir as mybir
    import concourse.tile as tile

    f16 = mybir.dt.float16
    f32 = mybir.dt.float32
    EXP = mybir.ActivationFunctionType.Exp

    nc = bacc.Bacc("TRN2", target_bir_lowering=False, debug=False,
                   num_devices=N_CORES)

    din = lambda n, s, dt=f16: nc.dram_tensor(n, s, dt, kind="ExternalInput").ap()
    compT_d = din("compT", [C, S])   # components[n,b] token-chan-major (host .T)
    posT_d = din("posT", [C, S])     # pos^T
    cfT_d = din("cfT", [C, S])       # content tokens^T
    cfc_d = din("cfc", [C, S])       # content real-channel-major (conv input)
    wk_d = din("wk", [C, C])         # Wkv[:, :C]
    wv_d = din("wv", [C, C])         # Wkv[:, C:]
    wq_d = din("wq", [C, C])
    wproj_d = din("wproj", [C, C])
    wcs_d = din("wcs", [C, C])       # Wconv^T rows 0:C   (s part)
    wccf_d = din("wccf", [C, C])     # Wconv^T rows C:2C  (cf part; 0 for n>0)
    bp4_d = din("bp4", [1, C])       # bproj/4
    bc4_d = din("bc4", [1, C])       # bconv/4
    out_p = nc.dram_tensor("out_p", [C, S], f16, kind="ExternalOutput").ap()

    # [C, C] weights as single-DMA [128, 4, C] chunked views
    wview = lambda d: d.rearrange("(k p) c -> p k c", p=128)
    tview = lambda t: t[:].rearrange("p (k c) -> p k c", k=4)

    with tile.TileContext(nc) as tc, ExitStack() as ctx:
        main = ctx.enter_context(tc.tile_pool(name="main", bufs=1))
        trans = ctx.enter_context(tc.tile_pool(name="trans", bufs=2))

        # ---- input DMAs (SP ring, in order of need) ----
        compT_raw = [main.tile([128, S], f16, tag=f"cr{j}", name=f"compTr{j}")
                     for j in range(4)]
        posT = [main.tile([128, S], f16, tag=f"pos{j}", name=f"posT{j}")
                for j in range(4)]
        for j in range(4):
            nc.sync.dma_start(compT_raw[j][:], compT_d[128 * j:128 * (j + 1), :])
            nc.sync.dma_start(posT[j][:], posT_d[128 * j:128 * (j + 1), :])
        wk = main.tile([128, 4 * C], f16, tag="wk")
        wv = main.tile([128, 4 * C], f16, tag="wv")
        wq = main.tile([128, 4 * C], f16, tag="wq")
        nc.sync.dma_start(tview(wk), wview(wk_d)[:, :, :])
        nc.sync.dma_start(tview(wv), wview(wv_d)[:, :, :])
        nc.sync.dma_start(tview(wq), wview(wq_d)[:, :, :])
        cfT = [main.tile([128, S], f16, tag=f"cfT{j}", name=f"cfT{j}")
               for j in range(4)]
        for j in range(4):
            nc.sync.dma_start(cfT[j][:], cfT_d[128 * j:128 * (j + 1), :])
        wproj = main.tile([128, 4 * C], f16, tag="wp")
        wcs = main.tile([128, 4 * C], f16, tag="wcs")
        wccf = main.tile([128, 4 * C], f16, tag="wccf")
        nc.sync.dma_start(tview(wproj), wview(wproj_d)[:, :, :])
        nc.sync.dma_start(tview(wcs), wview(wcs_d)[:, :, :])
        nc.sync.dma_start(tview(wccf), wview(wccf_d)[:, :, :])
        cfc = [main.tile([128, S], f16, tag=f"cfc{j}", name=f"cfc{j}")
               for j in range(4)]
        for j in range(4):
            nc.sync.dma_start(cfc[j][:], cfc_d[128 * j:128 * (j + 1), :])
        bp4 = main.tile([1, C], f16, tag="bp4")
        bc4 = main.tile([1, C], f16, tag="bc4")
        nc.sync.dma_start(bp4[:], bp4_d[:])
        nc.sync.dma_start(bc4[:], bc4_d[:])

        wk_v = wk[:].rearrange("p (k c) -> p k c", k=4)
        wv_v = wv[:].rearrange("p (k c) -> p k c", k=4)
        wq_v = wq[:].rearrange("p (k c) -> p k c", k=4)
        wproj_v = wproj[:].rearrange("p (k c) -> p k c", k=4)
        wcs_v = wcs[:].rearrange("p (k c) -> p k c", k=4)
        wccf_v = wccf[:].rearrange("p (k c) -> p k c", k=4)

        ones = main.tile([128, S], f16, tag="ones")
        nc.gpsimd.memset(ones[:], 1.0)
        ebias = main.tile([128, 1], f32, tag="ebias")
        nc.gpsimd.memset(ebias[:], EXP_BIAS)

        # ---- token adds ----
        # comp tokens: natural order (keys/values -- order irrelevant)
        tok = [main.tile([128, S], f16, tag=f"tok{j}", name=f"tok{j}")
               for j in range(4)]
        for j in range(4):
            nc.vector.tensor_add(tok[j][:], compT_raw[j][:], posT[j][:])
        # content tokens: parity-permuted columns (even tokens then odd)
        ctokT = [main.tile([128, S], f16, tag=f"ctokT{j}", name=f"ctokT{j}")
                 for j in range(4)]
        ev = lambda ap, par: ap.rearrange("p (a two) -> p a two", two=2)[
            :, :, par:par + 1]
        for j in range(4):
            for par in range(2):
                dst = ctokT[j][:, 512 * par:512 * (par + 1)].rearrange(
                    "p (a o) -> p a o", o=1)
                nc.vector.tensor_add(dst, ev(cfT[j][:], par), ev(posT[j][:], par))

        with tc.tile_pool(name="ps", bufs=1, space="PSUM") as ps:
            _scn = [0]

            def sct():
                _scn[0] += 1
                return ps.tile([128, S], f32, tag="sc", bufs=2,
                               name=f"sc{_scn[0]}")

            # ---- kT (k-dims major) ----
            kT = [main.tile([128, S], f16, tag=f"kT{j}", name=f"kT{j}")
                  for j in range(4)]
            for j in range(4):
                for tck in range(2):
                    acc = sct()
                    for k in range(4):
                        nc.tensor.matmul(acc[:, 0:512],
                                         wk_v[:, k, 128 * j:128 * (j + 1)],
                                         tok[k][:, 512 * tck:512 * (tck + 1)],
                                         start=(k == 0), stop=(k == 3))
                    nc.scalar.copy(kT[j][:, 512 * tck:512 * (tck + 1)],
                                   acc[:, 0:512])

            # ---- v (token-major, +ones col for rowsum) ----
            v_sb = [main.tile([128, 8 * 65], f16, tag=f"v{t}", name=f"v{t}")
                    for t in range(8)]
            for t in range(8):
                nc.gpsimd.tensor_copy(
                    v_sb[t][:].rearrange("p (h e) -> p h e", h=8)[:, :, 64:65],
                    ones[:, 0:8].rearrange("p (h w) -> p h w", w=1))
            for t in range(8):
                acc = sct()
                for k in range(4):
                    nc.tensor.matmul(acc[:, 0:512],
                                     tok[k][:, 128 * t:128 * (t + 1)],
                                     wv_v[:, k, :],
                                     start=(k == 0), stop=(k == 3))
                nc.scalar.copy(
                    v_sb[t][:].rearrange("p (h e) -> p h e", h=8)[:, :, 0:64],
                    acc[:, 0:512].rearrange("p (h d) -> p h d", h=8))

            # ---- qT (q-dims major, permuted token cols) ----
            qT = [main.tile([128, S], f16, tag=f"qT{j}", name=f"qT{j}")
                  for j in range(4)]
            for j in range(4):
                for tck in range(2):
                    acc = sct()
                    for k in range(4):
                        nc.tensor.matmul(acc[:, 0:512],
                                         wq_v[:, k, 128 * j:128 * (j + 1)],
                                         ctokT[k][:, 512 * tck:512 * (tck + 1)],
                                         start=(k == 0), stop=(k == 3))
                    nc.vector.tensor_copy(qT[j][:, 512 * tck:512 * (tck + 1)],
                                          acc[:, 0:512])
            # scale ctokT in place for the (result + ctok) merge: each of the
            # 4 component cores contributes a quarter of the ctok term
            for j in range(4):
                nc.vector.tensor_scalar_mul(ctokT[j][:], ctokT[j][:], 0.25)

            # ---- attention + interleaved cf-side conv ----
            pair = [main.tile([128, S], f16, tag=f"pair{j}", name=f"pair{j}")
                    for j in range(4)]
            outcf = [main.tile([128, S], f32, tag=f"ocf{oc}", name=f"ocf{oc}")
                     for oc in range(4)]

            def conv_cf(oc):
                # cf-half conv + quartered bconv for one oc chunk; fills
                # PE gaps while attention is ACT(exp)-bound
                acc = sct()
                for pc in range(2):
                    nc.tensor.matmul(acc[:, 512 * pc:512 * (pc + 1)],
                                     bc4[0:1, 128 * oc:128 * (oc + 1)],
                                     ones[0:1, 0:512],
                                     start=True, stop=False)
                    for k in range(4):
                        nc.tensor.matmul(acc[:, 512 * pc:512 * (pc + 1)],
                                         wccf_v[:, k, 128 * oc:128 * (oc + 1)],
                                         cfc[k][:, 512 * pc:512 * (pc + 1)],
                                         start=False, stop=(k == 3))
                nc.vector.tensor_copy(outcf[oc][:], acc[:])

            for h in range(NH):
                jq, row = h // 2, 64 * (h % 2)
                o_ps = ps.tile([65, S], f32, tag="o", bufs=2)
                for kt in range(8):
                    sc = sct()
                    for qc in range(2):
                        nc.tensor.matmul(
                            sc[:, 512 * qc:512 * (qc + 1)],
                            kT[jq][row:row + 64, 128 * kt:128 * (kt + 1)],
                            qT[jq][row:row + 64, 512 * qc:512 * (qc + 1)],
                            start=True, stop=True)
                    pt = trans.tile([128, S], f16, tag="pt", bufs=3,
                                    name=f"pt{h}_{kt}")
                    nc.scalar.activation(pt[:], sc[:], EXP, bias=ebias[:, 0:1],
                                         scale=SCALE)
                    for qc in range(2):
                        nc.tensor.matmul(
                            o_ps[:, 512 * qc:512 * (qc + 1)],
                            v_sb[kt][:].rearrange("p (h e) -> p h e", h=8)[:, h, :],
                            pt[:, 512 * qc:512 * (qc + 1)],
                            start=(kt == 0), stop=(kt == 7))
                # normalization: Z row 64 -> 1/Z -> broadcast -> scale rows 0:64
                zscr = trans.tile([1, S], f32, tag="zscr", bufs=2, name=f"zs{h}")
                zinv = trans.tile([1, S], f32, tag="zinv", bufs=2, name=f"zi{h}")
                zs2 = trans.tile([1, S], f32, tag="zs2", bufs=2, name=f"z2{h}")
                zb = trans.tile([64, S], f32, tag="zb", bufs=2, name=f"zb{h}")
                nc.vector.tensor_copy(zscr[0:1, :], o_ps[64:65, :])
                nc.vector.reciprocal_approx_accurate(
                    zinv[0:1, :], zscr[0:1, :], zs2[0:1, :])
                nc.gpsimd.partition_broadcast(zb[0:64, :], zinv[0:1, :])
                nc.vector.tensor_mul(pair[jq][row:row + 64, :],
                                     o_ps[0:64, :], zb[0:64, :])
                if h % 2 == 1:
                    # pair complete: merge the quartered ctok term
                    nc.vector.tensor_add(pair[jq][:], pair[jq][:], ctokT[jq][:])
                    conv_cf(jq)

            # ---- proj (s chunks stay in SBUF; rows are parity-ordered) ----
            s_sb = [main.tile([128, C], f16, tag=f"s{t}", name=f"s{t}")
                    for t in range(8)]
            for t in range(8):
                acc = sct()
                nc.tensor.matmul(acc[:, 0:512], ones[0:1, 0:128], bp4[0:1, :],
                                 start=True, stop=False)
                for j in range(4):
                    nc.tensor.matmul(acc[:, 0:512],
                                     pair[j][:, 128 * t:128 * (t + 1)],
                                     wproj_v[:, j, :],
                                     start=False, stop=(j == 3))
                nc.scalar.copy(s_sb[t][:], acc[:, 0:512])

            # ---- s-side conv + combine + out ----
            out_sb = [main.tile([128, S], f16, tag=f"ob{oc}", name=f"ob{oc}")
                      for oc in range(4)]
            for pc in range(2):
                for oc in range(4):
                    acc = sct()
                    for k in range(4):
                        nc.tensor.matmul(acc[:, 0:512],
                                         wcs_v[:, k, 128 * oc:128 * (oc + 1)],
                                         s_sb[4 * pc + k][:],
                                         start=(k == 0), stop=(k == 3))
                    nc.vector.tensor_add(
                        out_sb[oc][:, 512 * pc:512 * (pc + 1)],
                        acc[:, 0:512],
                        outcf[oc][:, 512 * pc:512 * (pc + 1)])
                    if pc == 1:
                        nc.sync.dma_start(out_p[128 * oc:128 * (oc + 1), :],
                                          out_sb[oc][:])

    nc.compile()
    _CACHE["nc"] = nc
    return nc


def _shard_inputs(content_feat, components, pos_emb, Wq, Wkv, Wproj, bproj,
                  Wconv, bconv):
    h = np.float16
    ca = np.ascontiguousarray
    posT = ca(pos_emb.reshape(S, C).T, dtype=h)
    wk = ca(Wkv[:, :C], dtype=h)
    wv = ca(Wkv[:, C:], dtype=h)
    wq = ca(Wq, dtype=h)
    wp = ca(Wproj, dtype=h)
    wcT = ca(Wconv.T, dtype=h)
    wcs = ca(wcT[:C])
    wccf_first = ca(wcT[C:])
    wccf_rest = np.zeros_like(wccf_first)
    bp4 = ca(bproj.reshape(1, C) / 4.0, dtype=h)
    bc4 = ca(bconv.reshape(1, C) / 4.0, dtype=h)
    in_maps = []
    for core in range(N_CORES):
        b, n = core // 4, core % 4
        in_maps.append({
            "compT": ca(components[n, b].reshape(S, C).T, dtype=h),
            "posT": posT,
            "cfT": ca(content_feat[b].reshape(S, C).T, dtype=h),
            "cfc": ca(content_feat[b].reshape(C, S), dtype=h),
            "wk": wk, "wv": wv, "wq": wq, "wproj": wp,
            "wcs": wcs,
            "wccf": wccf_first if n == 0 else wccf_rest,
            "bp4": bp4, "bc4": bc4,
        })
    return in_maps


def _run(trace=False, **inputs):
    from concourse.bass_utils import run_bass_kernel_spmd

    nc = _build()
    in_maps = _shard_inputs(**inputs)
    res = run_bass_kernel_spmd(nc, in_maps, list(range(N_CORES)), trace=trace)
    outs = [res.results[i]["out_p"].astype(np.float32) for i in range(N_CORES)]
    out = np.stack([outs[0] + outs[1] + outs[2] + outs[3],
                    outs[4] + outs[5] + outs[6] + outs[7]], axis=0)
    return out.reshape(B, C, H, W).astype(np.float32), res


def kernel(**inputs):
    out, _ = _run(trace=False, **inputs)
    return out


# revision 18
# speedup vs baseline: 1.1305x; 1.0405x over previous
"""Trainium2 Bass kernel for nn_Attention_54391465836966.

Math (per batch b, component n; one (b, n) pair per core):
  ctok = content_feat[b].reshape(S,C) + pos           # raw reshape tokens
  comp_tok = components[n,b].reshape(S,C) + pos
  q = ctok @ Wq ; k,v = comp_tok @ Wkv (split)
  per head h: P' = exp(scale*q k^T - 12); o_h = (P' @ v) / rowsum(P')
  result = sum_n o_n ; s = (result + ctok) @ Wproj + bproj
  out = Wconv[:, :512] . s2d + Wconv[:, 512:] . cf2d + bconv
    (s2d = raw [C, S] view of the token-major s buffer)

All device data is fp16 (f32 PSUM accumulation); exp carries a constant
-12 bias so probabilities fit fp16 (softmax is invariant to it).  Host
passes token-channel-major transposes (comp^T, cf^T, pos^T) and Wconv^T,
so the kernel does zero PE transposes.  Query tokens are parity-permuted
(even tokens first) end-to-end: the proj output tiles then directly ARE
the raw-reshape s2d chunks the conv needs, removing the DRAM round-trip.
The (result + ctok) constant term rides the attention output as
+0.25*ctok^T per core (host sums 4 component partials per batch); the
bproj term's conv contribution is the rank-1 update colsum(Wconv_s) x
bproj/4, folded into the cf-side conv.

Schedule: the attention exp stream (ACT engine) is the longest chain, so
everything else is threaded through its gaps -- v/kT/qT setup matmuls
fill heads 0-3 and the cf-side conv chunks fill heads 4-7, keeping exp
contiguous from ~14us on.  Normalization copies o out of PSUM once
(freeing the single o bank pair), takes 1/Z via the fast DVE reciprocal
from the ones-column row, and scales into head-pair tiles consumed by
the proj/conv tail.
"""
import sys

sys.path.insert(0, "/opt/trn_rl_repo")

import numpy as np

N_CORES = 8
B, C, H, W = 2, 512, 32, 32
S = H * W  # 1024
NH, HD = 8, 64
SCALE = HD ** -0.5
EXP_BIAS = -12.0

_CACHE = {}


def _build():
    if "nc" in _CACHE:
        return _CACHE["nc"]
    from contextlib import ExitStack

    import concourse.bacc as bacc
    import concourse.mybir as mybir
    import concourse.tile as tile

    f16 = mybir.dt.float16
    f32 = mybir.dt.float32
    EXP = mybir.ActivationFunctionType.Exp

    nc = bacc.Bacc("TRN2", target_bir_lowering=False, debug=False,
                   num_devices=N_CORES)

    din = lambda n, s, dt=f16: nc.dram_tensor(n, s, dt, kind="ExternalInput").ap()
    compT_d = din("compT", [C, S])   # components[n,b] token-chan-major (host .T)
    posT_d = din("posT", [C, S])     # pos^T
    cfT_d = din("cfT", [C, S])       # content tokens^T
    cfc_d = din("cfc", [C, S])       # content real-channel-major (conv input)
    wk_d = din("wk", [C, C])         # Wkv[:, :C]
    wv_d = din("wv", [C, C])         # Wkv[:, C:]
    wq_d = din("wq", [C, C])
    wproj_d = din("wproj", [C, C])
    wcs_d = din("wcs", [C, C])       # Wconv^T rows 0:C   (s part)
    wccf_d = din("wccf", [C, C])     # Wconv^T rows C:2C  (cf part; 0 for n>0)
    bp4_d = din("bp4", [1, C])       # bproj/4
    bc4_d = din("bc4", [1, C])       # bconv/4
    wcsum_d = din("wcsum", [1, C])   # colsum of Wconv s-half (rank-1 bias term)
    out_p = nc.dram_tensor("out_p", [C, S], f16, kind="ExternalOutput").ap()

    wview = lambda d: d.rearrange("(k p) c -> p k c", p=128)
    tview = lambda t: t[:].rearrange("p (k c) -> p k c", k=4)

    with tile.TileContext(nc) as tc, ExitStack() as ctx:
        main = ctx.enter_context(tc.tile_pool(name="main", bufs=1))
        trans = ctx.enter_context(tc.tile_pool(name="trans", bufs=2))

        # ---- input DMAs (SP ring, in order of first use) ----
        compT_raw = [main.tile([128, S], f16, tag=f"cr{j}", name=f"compTr{j}")
                     for j in range(4)]
        posT = [main.tile([128, S], f16, tag=f"pos{j}", name=f"posT{j}")
                for j in range(4)]
        for j in range(4):
            nc.sync.dma_start(compT_raw[j][:], compT_d[128 * j:128 * (j + 1), :])
            nc.sync.dma_start(posT[j][:], posT_d[128 * j:128 * (j + 1), :])
        cfT = [main.tile([128, S], f16, tag=f"cfT{j}", name=f"cfT{j}")
               for j in range(4)]
        for j in range(4):
            nc.sync.dma_start(cfT[j][:], cfT_d[128 * j:128 * (j + 1), :])
        wk = main.tile([128, 4 * C], f16, tag="wk")
        wq = main.tile([128, 4 * C], f16, tag="wq")
        wv = main.tile([128, 4 * C], f16, tag="wv")
        nc.sync.dma_start(tview(wk), wview(wk_d)[:, :, :])
        nc.sync.dma_start(tview(wq), wview(wq_d)[:, :, :])
        nc.sync.dma_start(tview(wv), wview(wv_d)[:, :, :])
        wproj = main.tile([128, 4 * C], f16, tag="wp")
        wcs = main.tile([128, 4 * C], f16, tag="wcs")
        wccf = main.tile([128, 4 * C], f16, tag="wccf")
        nc.sync.dma_start(tview(wproj), wview(wproj_d)[:, :, :])
        nc.sync.dma_start(tview(wcs), wview(wcs_d)[:, :, :])
        nc.sync.dma_start(tview(wccf), wview(wccf_d)[:, :, :])
        cfc = [main.tile([128, S], f16, tag=f"cfc{j}", name=f"cfc{j}")
               for j in range(4)]
        for j in range(4):
            nc.sync.dma_start(cfc[j][:], cfc_d[128 * j:128 * (j + 1), :])
        bp4 = main.tile([1, C], f16, tag="bp4")
        bc4 = main.tile([1, C], f16, tag="bc4")
        wcsum = main.tile([1, C], f16, tag="wcsum")
        nc.sync.dma_start(bp4[:], bp4_d[:])
        nc.sync.dma_start(bc4[:], bc4_d[:])
        nc.sync.dma_start(wcsum[:], wcsum_d[:])

        wk_v, wq_v, wv_v = tview(wk), tview(wq), tview(wv)
        wproj_v, wcs_v, wccf_v = tview(wproj), tview(wcs), tview(wccf)

        ones = main.tile([128, S], f16, tag="ones")
        nc.gpsimd.memset(ones[:], 1.0)
        ebias = main.tile([128, 1], f32, tag="ebias")
        nc.gpsimd.memset(ebias[:], EXP_BIAS)

        # ---- token adds ----
        tok = [main.tile([128, S], f16, tag=f"tok{j}", name=f"tok{j}")
               for j in range(4)]
        for j in range(4):
            nc.vector.tensor_add(tok[j][:], compT_raw[j][:], posT[j][:])
        # content tokens: parity-permuted columns (even tokens then odd)
        ctokT = [main.tile([128, S], f16, tag=f"ctokT{j}", name=f"ctokT{j}")
                 for j in range(4)]
        ev = lambda ap, par: ap.rearrange("p (a two) -> p a two", two=2)[
            :, :, par:par + 1]
        for j in range(4):
            for par in range(2):
                dst = ctokT[j][:, 512 * par:512 * (par + 1)].rearrange(
                    "p (a o) -> p a o", o=1)
                nc.vector.tensor_add(dst, ev(cfT[j][:], par), ev(posT[j][:], par))

        kT = [main.tile([128, S], f16, tag=f"kT{j}", name=f"kT{j}")
              for j in range(4)]
        qT = [main.tile([128, S], f16, tag=f"qT{j}", name=f"qT{j}")
              for j in range(4)]
        v_sb = [main.tile([128, 8 * 65], f16, tag=f"v{t}", name=f"v{t}")
                for t in range(8)]
        vv = lambda t: v_sb[t][:].rearrange("p (h e) -> p h e", h=8)
        for t in range(8):
            # ones column LAST (e=64): Z lands on o_ps partition 64 (aligned)
            nc.gpsimd.tensor_copy(vv(t)[:, :, 64:65],
                                  ones[:, 0:8].rearrange("p (h w) -> p h w", w=1))
        pair = [main.tile([128, S], f16, tag=f"pair{j}", name=f"pair{j}")
                for j in range(4)]
        outcf = [main.tile([128, S], f32, tag=f"ocf{oc}", name=f"ocf{oc}")
                 for oc in range(4)]
        s_sb = [main.tile([128, C], f16, tag=f"s{t}", name=f"s{t}")
                for t in range(8)]
        out_sb = [main.tile([128, S], f16, tag=f"ob{oc}", name=f"ob{oc}")
                  for oc in range(4)]

        with tc.tile_pool(name="ps", bufs=1, space="PSUM") as ps:
            _n = [0]

            def pst(tag, shape, bufs):
                _n[0] += 1
                return ps.tile(shape, f32, tag=tag, bufs=bufs,
                               name=f"{tag}{_n[0]}")

            sct = lambda: pst("sc", [128, S], 2)

            # ---- filler emitters (run inside attention's ACT-bound gaps) --
            def emit_kq(dst, wsrc, act, j, tck):
                acc = sct()
                for k in range(4):
                    nc.tensor.matmul(acc[:, 0:512],
                                     wsrc[:, k, 128 * j:128 * (j + 1)],
                                     act[k][:, 512 * tck:512 * (tck + 1)],
                                     start=(k == 0), stop=(k == 3))
                if dst is qT:
                    nc.vector.tensor_copy(dst[j][:, 512 * tck:512 * (tck + 1)],
                                          acc[:, 0:512])
                else:
                    nc.scalar.copy(dst[j][:, 512 * tck:512 * (tck + 1)],
                                   acc[:, 0:512])

            def emit_v(t):
                acc = sct()
                for k in range(4):
                    nc.tensor.matmul(acc[:, 0:512],
                                     tok[k][:, 128 * t:128 * (t + 1)],
                                     wv_v[:, k, :],
                                     start=(k == 0), stop=(k == 3))
                nc.vector.tensor_copy(
                    vv(t)[:, :, 0:64],
                    acc[:, 0:512].rearrange("p (h d) -> p h d", h=8))

            def emit_cc(oc):
                # cf-half conv + quartered bconv + rank-1 bproj term
                acc = sct()
                for pc in range(2):
                    half = acc[:, 512 * pc:512 * (pc + 1)]
                    nc.tensor.matmul(half, bc4[0:1, 128 * oc:128 * (oc + 1)],
                                     ones[0:1, 0:512], start=True, stop=False)
                    nc.tensor.matmul(half, wcsum[0:1, 128 * oc:128 * (oc + 1)],
                                     bp4[0:1, :], start=False, stop=False)
                    for k in range(4):
                        nc.tensor.matmul(half,
                                         wccf_v[:, k, 128 * oc:128 * (oc + 1)],
                                         cfc[k][:, 512 * pc:512 * (pc + 1)],
                                         start=False, stop=(k == 3))
                nc.vector.tensor_copy(outcf[oc][:], acc[:])

            fillers = []
            for j in (1, 2, 3):
                for tck in range(2):
                    fillers.append(lambda j=j, t=tck: emit_kq(kT, wk_v, tok, j, t))
                for tck in range(2):
                    fillers.append(lambda j=j, t=tck: emit_kq(qT, wq_v, ctokT,
                                                              j, t))
            fillers.reverse()  # pop() takes them in the listed order

            # ---- lead-in: kT0 / qT0 ----
            for tck in range(2):
                emit_kq(kT, wk_v, tok, 0, tck)
            for tck in range(2):
                emit_kq(qT, wq_v, ctokT, 0, tck)

            # ---- attention: contiguous exp stream + fillers ----
            def norm(h, o_ps):
                jq, row = h // 2, 64 * (h % 2)
                zscr = trans.tile([1, S], f32, tag="zscr", bufs=2, name=f"zs{h}")
                zinv = trans.tile([1, S], f32, tag="zinv", bufs=2, name=f"zi{h}")
                zb = trans.tile([64, S], f32, tag="zb", bufs=2, name=f"zb{h}")
                nc.vector.tensor_copy(zscr[0:1, :], o_ps[64:65, :])
                nc.vector.reciprocal_approx_fast(zinv[0:1, :], zscr[0:1, :])
                nc.gpsimd.partition_broadcast(zb[0:64, :], zinv[0:1, :])
                nc.vector.tensor_mul(pair[jq][row:row + 64, :],
                                     o_ps[0:64, :], zb[0:64, :])

            merge = lambda j: nc.vector.tensor_add(pair[j][:], pair[j][:],
                                                   ctokT[j][:])
            for h in range(NH):
                jq, row = h // 2, 64 * (h % 2)
                if h == 4:
                    # all qT consumers of ctokT are done: scale in place; each
                    # component core contributes a quarter of the ctok term
                    for j in range(4):
                        nc.vector.tensor_scalar_mul(ctokT[j][:], ctokT[j][:],
                                                    0.25)
                o_ps = pst("o", [65, S], 2)
                for kt in range(8):
                    sc = sct()
                    for qc in range(2):
                        nc.tensor.matmul(
                            sc[:, 512 * qc:512 * (qc + 1)],
                            kT[jq][row:row + 64, 128 * kt:128 * (kt + 1)],
                            qT[jq][row:row + 64, 512 * qc:512 * (qc + 1)],
                            start=True, stop=True)
                    pt = trans.tile([128, S], f16, tag="pt", bufs=3,
                                    name=f"pt{h}_{kt}")
                    nc.scalar.activation(pt[:], sc[:], EXP, bias=ebias[:, 0:1],
                                         scale=SCALE)
                    if h == 0:
                        emit_v(kt)  # v tile needed by this very o step
                    elif kt % 4 == 1 and fillers:
                        fillers.pop()()
                    elif kt % 4 == 3 and fillers:
                        fillers.pop()()
                    for qc in range(2):
                        nc.tensor.matmul(
                            o_ps[:, 512 * qc:512 * (qc + 1)],
                            vv(kt)[:, h, :],
                            pt[:, 512 * qc:512 * (qc + 1)],
                            start=(kt == 0), stop=(kt == 7))
                norm(h, o_ps)
                if h >= 5:
                    merge(h - 5)  # pair 0..2 merges ride the attention tail
            while fillers:
                fillers.pop()()
            merge(3)
            # cf-side conv overlaps the h7 normalization / merges
            for oc in range(4):
                emit_cc(oc)

            # ---- proj (s chunks stay in SBUF; rows are parity-ordered) ----
            def emit_proj(t):
                acc = sct()
                for j in range(4):
                    nc.tensor.matmul(acc[:, 0:512],
                                     pair[j][:, 128 * t:128 * (t + 1)],
                                     wproj_v[:, j, :],
                                     start=(j == 0), stop=(j == 3))
                nc.scalar.copy(s_sb[t][:], acc[:, 0:512])

            def emit_cs(oc, pc):
                acc = sct()
                for k in range(4):
                    nc.tensor.matmul(acc[:, 0:512],
                                     wcs_v[:, k, 128 * oc:128 * (oc + 1)],
                                     s_sb[4 * pc + k][:],
                                     start=(k == 0), stop=(k == 3))
                nc.vector.tensor_add(out_sb[oc][:, 512 * pc:512 * (pc + 1)],
                                     acc[:, 0:512],
                                     outcf[oc][:, 512 * pc:512 * (pc + 1)])
                if pc == 1:
                    nc.sync.dma_start(out_p[128 * oc:128 * (oc + 1), :],
                                      out_sb[oc][:])

            for t in range(4):
                emit_proj(t)
            for t in range(4, 8):
                emit_proj(t)
                emit_cs(t - 4, 0)
            for oc in range(4):
                emit_cs(oc, 1)

    nc.compile()
    _CACHE["nc"] = nc
    return nc


def _shard_inputs(content_feat, components, pos_emb, Wq, Wkv, Wproj, bproj,
                  Wconv, bconv):
    h = np.float16
    ca = np.ascontiguousarray
    posT = ca(pos_emb.reshape(S, C).T, dtype=h)
    wk = ca(Wkv[:, :C], dtype=h)
    wv = ca(Wkv[:, C:], dtype=h)
    wq = ca(Wq, dtype=h)
    wp = ca(Wproj, dtype=h)
    wcT = ca(Wconv.T, dtype=h)
    wcs = ca(wcT[:C])
    wccf_first = ca(wcT[C:])
    wccf_rest = np.zeros_like(wccf_first)
    bp4 = ca(bproj.reshape(1, C) / 4.0, dtype=h)
    bc4 = ca(bconv.reshape(1, C) / 4.0, dtype=h)
    wcsum = ca(Wconv[:, :C].sum(axis=1).reshape(1, C), dtype=h)
    in_maps = []
    for core in range(N_CORES):
        b, n = core // 4, core % 4
        in_maps.append({
            "compT": ca(components[n, b].reshape(S, C).T, dtype=h),
            "posT": posT,
            "cfT": ca(content_feat[b].reshape(S, C).T, dtype=h),
            "cfc": ca(content_feat[b].reshape(C, S), dtype=h),
            "wk": wk, "wv": wv, "wq": wq, "wproj": wp,
            "wcs": wcs,
            "wccf": wccf_first if n == 0 else wccf_rest,
            "bp4": bp4, "bc4": bc4, "wcsum": wcsum,
        })
    return in_maps


def _run(trace=False, **inputs):
    from concourse.bass_utils import run_bass_kernel_spmd

    nc = _build()
    in_maps = _shard_inputs(**inputs)
    res = run_bass_kernel_spmd(nc, in_maps, list(range(N_CORES)), trace=trace)
    outs = [res.results[i]["out_p"].astype(np.float32) for i in range(N_CORES)]
    out = np.stack([outs[0] + outs[1] + outs[2] + outs[3],
                    outs[4] + outs[5] + outs[6] + outs[7]], axis=0)
    return out.reshape(B, C, H, W).astype(np.float32), res


def kernel(**inputs):
    out, _ = _run(trace=False, **inputs)
    return out


# revision 22
# speedup vs baseline: 1.1975x; 1.0593x over previous
"""Trainium2 Bass kernel for nn_Attention_54391465836966.

Math (per batch b, component n; one (b, n) pair per core):
  ctok = content_feat[b].reshape(S,C) + pos           # raw reshape tokens
  comp_tok = components[n,b].reshape(S,C) + pos
  q = ctok @ Wq ; k,v = comp_tok @ Wkv (split)
  per head h: P' = exp(scale*q k^T - 12); o_h = (P' @ v) / rowsum(P')
  result = sum_n o_n ; s = (result + ctok) @ Wproj + bproj
  out = Wconv[:, :512] . s2d + Wconv[:, 512:] . cf2d + bconv
    (s2d = raw [C, S] view of the token-major s buffer)

All device data is fp16 (f32 PSUM accumulation); exp carries a constant
-12 bias so probabilities fit fp16 (softmax is invariant to it).  Host
passes token-channel-major transposes (comp^T, cf^T, pos^T) and Wconv^T,
so the kernel does zero PE transposes.  Query tokens are parity-permuted
(even tokens first) end-to-end: the proj output tiles then directly ARE
the raw-reshape s2d chunks the conv needs, removing the DRAM round-trip.
The (result + ctok) constant term rides the attention output as
+0.25*ctok^T per core (host sums 4 component partials per batch); the
bproj term's conv contribution is the rank-1 update colsum(Wconv_s) x
bproj/4, folded into the cf-side conv.

Schedule: the attention exp stream (ACT engine) is the longest chain, so
everything else is threaded through its gaps -- v/kT/qT setup matmuls
fill heads 0-3 and the cf-side conv chunks fill heads 4-7, keeping exp
contiguous from ~14us on.  Normalization copies o out of PSUM once
(freeing the single o bank pair), takes 1/Z via the fast DVE reciprocal
from the ones-column row, and scales into head-pair tiles consumed by
the proj/conv tail.
"""
import sys

sys.path.insert(0, "/opt/trn_rl_repo")

import numpy as np

N_CORES = 8
B, C, H, W = 2, 512, 32, 32
S = H * W  # 1024
NH, HD = 8, 64
SCALE = HD ** -0.5
EXP_BIAS = -12.0

_CACHE = {}


def _build():
    if "nc" in _CACHE:
        return _CACHE["nc"]
    from contextlib import ExitStack

    import concourse.bacc as bacc
    import concourse.mybir as mybir
    import concourse.tile as tile

    f16 = mybir.dt.float16
    f32 = mybir.dt.float32
    EXP = mybir.ActivationFunctionType.Exp

    nc = bacc.Bacc("TRN2", target_bir_lowering=False, debug=False,
                   num_devices=N_CORES)

    din = lambda n, s, dt=f16: nc.dram_tensor(n, s, dt, kind="ExternalInput").ap()
    compT_d = din("compT", [C, S])   # components[n,b] token-chan-major (host .T)
    posT_d = din("posT", [C, S])     # pos^T
    cfT_d = din("cfT", [C, S])       # content tokens^T
    cfc_d = din("cfc", [C, S])       # content real-channel-major (conv input)
    wk_d = din("wk", [C, C])         # Wkv[:, :C]
    wv_d = din("wv", [C, C])         # Wkv[:, C:]
    wq_d = din("wq", [C, C])
    wproj_d = din("wproj", [C, C])
    wcs_d = din("wcs", [C, C])       # Wconv^T rows 0:C   (s part)
    wccf_d = din("wccf", [C, C])     # Wconv^T rows C:2C  (cf part; 0 for n>0)
    bp4_d = din("bp4", [1, C])       # bproj/4
    bc4_d = din("bc4", [1, C])       # bconv/4
    wcsum_d = din("wcsum", [1, C])   # colsum of Wconv s-half (rank-1 bias term)
    out_p = nc.dram_tensor("out_p", [C, S], f16, kind="ExternalOutput").ap()

    wview = lambda d: d.rearrange("(k p) c -> p k c", p=128)
    tview = lambda t: t[:].rearrange("p (k c) -> p k c", k=4)

    with tile.TileContext(nc) as tc, ExitStack() as ctx:
        main = ctx.enter_context(tc.tile_pool(name="main", bufs=1))
        trans = ctx.enter_context(tc.tile_pool(name="trans", bufs=2))

        # ---- input DMAs (SP ring, in order of first use) ----
        compT_raw = [main.tile([128, S], f16, tag=f"cr{j}", name=f"compTr{j}")
                     for j in range(4)]
        posT = [main.tile([128, S], f16, tag=f"pos{j}", name=f"posT{j}")
                for j in range(4)]
        for j in range(4):
            nc.sync.dma_start(compT_raw[j][:], compT_d[128 * j:128 * (j + 1), :])
            nc.sync.dma_start(posT[j][:], posT_d[128 * j:128 * (j + 1), :])
        wk = main.tile([128, 4 * C], f16, tag="wk")
        wq = main.tile([128, 4 * C], f16, tag="wq")
        wv = main.tile([128, 4 * C], f16, tag="wv")
        nc.sync.dma_start(tview(wk), wview(wk_d)[:, :, :])
        nc.sync.dma_start(tview(wv), wview(wv_d)[:, :, :])
        cfT = [main.tile([128, S], f16, tag=f"cfT{j}", name=f"cfT{j}")
               for j in range(4)]
        for j in range(4):
            nc.sync.dma_start(cfT[j][:], cfT_d[128 * j:128 * (j + 1), :])
        nc.sync.dma_start(tview(wq), wview(wq_d)[:, :, :])
        wproj = main.tile([128, 4 * C], f16, tag="wp")
        wcs = main.tile([128, 4 * C], f16, tag="wcs")
        wccf = main.tile([128, 4 * C], f16, tag="wccf")
        nc.sync.dma_start(tview(wproj), wview(wproj_d)[:, :, :])
        nc.sync.dma_start(tview(wcs), wview(wcs_d)[:, :, :])
        nc.sync.dma_start(tview(wccf), wview(wccf_d)[:, :, :])
        cfc = [main.tile([128, S], f16, tag=f"cfc{j}", name=f"cfc{j}")
               for j in range(4)]
        for j in range(4):
            nc.sync.dma_start(cfc[j][:], cfc_d[128 * j:128 * (j + 1), :])
        bp4 = main.tile([1, C], f16, tag="bp4")
        bc4 = main.tile([1, C], f16, tag="bc4")
        wcsum = main.tile([1, C], f16, tag="wcsum")
        nc.sync.dma_start(bp4[:], bp4_d[:])
        nc.sync.dma_start(bc4[:], bc4_d[:])
        nc.sync.dma_start(wcsum[:], wcsum_d[:])

        wk_v, wq_v, wv_v = tview(wk), tview(wq), tview(wv)
        wproj_v, wcs_v, wccf_v = tview(wproj), tview(wcs), tview(wccf)

        ones = main.tile([128, S], f16, tag="ones")
        nc.gpsimd.memset(ones[:], 1.0)
        ebias = main.tile([128, 1], f32, tag="ebias")
        nc.gpsimd.memset(ebias[:], EXP_BIAS)

        # ---- token adds ----
        tok = [main.tile([128, S], f16, tag=f"tok{j}", name=f"tok{j}")
               for j in range(4)]
        for j in range(4):
            nc.vector.tensor_add(tok[j][:], compT_raw[j][:], posT[j][:])
        # content tokens: parity-permuted columns (even tokens then odd)
        ctokT = [main.tile([128, S], f16, tag=f"ctokT{j}", name=f"ctokT{j}")
                 for j in range(4)]
        ev = lambda ap, par: ap.rearrange("p (a two) -> p a two", two=2)[
            :, :, par:par + 1]
        for j in range(4):
            for par in range(2):
                dst = ctokT[j][:, 512 * par:512 * (par + 1)].rearrange(
                    "p (a o) -> p a o", o=1)
                nc.vector.tensor_add(dst, ev(cfT[j][:], par), ev(posT[j][:], par))

        kT = [main.tile([128, S], f16, tag=f"kT{j}", name=f"kT{j}")
              for j in range(4)]
        qT = [main.tile([128, S], f16, tag=f"qT{j}", name=f"qT{j}")
              for j in range(4)]
        v_sb = [main.tile([128, 8 * 65], f16, tag=f"v{t}", name=f"v{t}")
                for t in range(8)]
        vv = lambda t: v_sb[t][:].rearrange("p (h e) -> p h e", h=8)
        for t in range(8):
            # ones column LAST (e=64): Z lands on o_ps partition 64 (aligned)
            nc.gpsimd.tensor_copy(vv(t)[:, :, 64:65],
                                  ones[:, 0:8].rearrange("p (h w) -> p h w", w=1))
        pair = [main.tile([128, S], f16, tag=f"pair{j}", name=f"pair{j}")
                for j in range(4)]
        outcf = [main.tile([128, S], f32, tag=f"ocf{oc}", name=f"ocf{oc}")
                 for oc in range(4)]
        s_sb = [main.tile([128, C], f16, tag=f"s{t}", name=f"s{t}")
                for t in range(8)]
        out_sb = [main.tile([128, S], f16, tag=f"ob{oc}", name=f"ob{oc}")
                  for oc in range(4)]

        with tc.tile_pool(name="ps", bufs=1, space="PSUM") as ps:
            _n = [0]

            def pst(tag, shape, bufs):
                _n[0] += 1
                return ps.tile(shape, f32, tag=tag, bufs=bufs,
                               name=f"{tag}{_n[0]}")

            sct = lambda: pst("sc", [128, S], 2)

            # ---- filler emitters (run inside attention's ACT-bound gaps) --
            def emit_kq(dst, wsrc, act, j, tck):
                acc = sct()
                for k in range(4):
                    nc.tensor.matmul(acc[:, 0:512],
                                     wsrc[:, k, 128 * j:128 * (j + 1)],
                                     act[k][:, 512 * tck:512 * (tck + 1)],
                                     start=(k == 0), stop=(k == 3))
                # both on DVE: the ACT engine must stay exp-only in attention
                nc.vector.tensor_copy(dst[j][:, 512 * tck:512 * (tck + 1)],
                                      acc[:, 0:512])

            def emit_v(t):
                acc = sct()
                for k in range(4):
                    nc.tensor.matmul(acc[:, 0:512],
                                     tok[k][:, 128 * t:128 * (t + 1)],
                                     wv_v[:, k, :],
                                     start=(k == 0), stop=(k == 3))
                nc.vector.tensor_copy(
                    vv(t)[:, :, 0:64],
                    acc[:, 0:512].rearrange("p (h d) -> p h d", h=8))

            def emit_cc(oc):
                # cf-half conv + quartered bconv + rank-1 bproj term
                acc = sct()
                for pc in range(2):
                    half = acc[:, 512 * pc:512 * (pc + 1)]
                    nc.tensor.matmul(half, bc4[0:1, 128 * oc:128 * (oc + 1)],
                                     ones[0:1, 0:512], start=True, stop=False)
                    nc.tensor.matmul(half, wcsum[0:1, 128 * oc:128 * (oc + 1)],
                                     bp4[0:1, :], start=False, stop=False)
                    for k in range(4):
                        nc.tensor.matmul(half,
                                         wccf_v[:, k, 128 * oc:128 * (oc + 1)],
                                         cfc[k][:, 512 * pc:512 * (pc + 1)],
                                         start=False, stop=(k == 3))
                nc.vector.tensor_copy(outcf[oc][:], acc[:])

            # ---- lead-in: kT0, first v tiles, qT0 ----
            emit_kq(kT, wk_v, tok, 0, 0)
            emit_kq(kT, wk_v, tok, 0, 1)
            for t in range(4):
                emit_v(t)
            emit_kq(qT, wq_v, ctokT, 0, 0)
            emit_kq(qT, wq_v, ctokT, 0, 1)

            # ---- attention, software-pipelined one head deep ----
            # Block h emits head h's scores+exp (the ACT critical stream)
            # interleaved per-kt with head h-1's o matmuls (whose exps
            # finished a full block ago -> PE never waits on ACT) plus one
            # filler group; a drain block finishes head 7.
            def norm(h, o_ps):
                jq, row = h // 2, 64 * (h % 2)
                zscr = trans.tile([1, S], f32, tag="zscr", bufs=2, name=f"zs{h}")
                zinv = trans.tile([1, S], f32, tag="zinv", bufs=2, name=f"zi{h}")
                zb = trans.tile([64, S], f32, tag="zb", bufs=2, name=f"zb{h}")
                nc.vector.tensor_copy(zscr[0:1, :], o_ps[64:65, :])
                nc.vector.reciprocal_approx_fast(zinv[0:1, :], zscr[0:1, :])
                nc.gpsimd.partition_broadcast(zb[0:64, :], zinv[0:1, :])
                nc.vector.tensor_mul(pair[jq][row:row + 64, :],
                                     o_ps[0:64, :], zb[0:64, :])

            merge = lambda j: nc.vector.tensor_add(pair[j][:], pair[j][:],
                                                   ctokT[j][:])
            fill_sched = {
                0: [lambda t=t: emit_v(t) for t in range(4, 8)],
                1: [lambda tc=tc: emit_kq(kT, wk_v, tok, 1, tc) for tc in (0, 1)]
                 + [lambda tc=tc: emit_kq(qT, wq_v, ctokT, 1, tc) for tc in (0, 1)],
                2: [lambda tc=tc: emit_kq(kT, wk_v, tok, 2, tc) for tc in (0, 1)]
                 + [lambda tc=tc: emit_kq(qT, wq_v, ctokT, 2, tc) for tc in (0, 1)],
                3: [lambda tc=tc: emit_kq(kT, wk_v, tok, 3, tc) for tc in (0, 1)]
                 + [lambda tc=tc: emit_kq(qT, wq_v, ctokT, 3, tc) for tc in (0, 1)],
                4: [lambda: emit_cc(0)],
                5: [lambda: emit_cc(1)],
                6: [lambda: emit_cc(2)],
            }
            pts = {}
            o_tiles = {}

            def emit_sc(h, kt):
                jq, row = h // 2, 64 * (h % 2)
                sc = sct()
                for qc in range(2):
                    nc.tensor.matmul(
                        sc[:, 512 * qc:512 * (qc + 1)],
                        kT[jq][row:row + 64, 128 * kt:128 * (kt + 1)],
                        qT[jq][row:row + 64, 512 * qc:512 * (qc + 1)],
                        start=True, stop=True)
                pt = trans.tile([128, S], f16, tag="pt", bufs=12,
                                name=f"pt{h}_{kt}")
                nc.scalar.activation(pt[:], sc[:], EXP, bias=ebias[:, 0:1],
                                     scale=SCALE)
                pts[(h, kt)] = pt

            def emit_o(h, kt):
                if kt == 0:
                    o_tiles[h] = pst("o", [65, S], 2)
                o_ps = o_tiles[h]
                for qc in range(2):
                    nc.tensor.matmul(
                        o_ps[:, 512 * qc:512 * (qc + 1)],
                        vv(kt)[:, h, :],
                        pts[(h, kt)][:, 512 * qc:512 * (qc + 1)],
                        start=(kt == 0), stop=(kt == 7))
                if kt == 7:
                    del pts[(h, kt)]

            for h in range(NH + 1):
                if h == 4:
                    # all qT consumers of ctokT are done: scale in place; each
                    # component core contributes a quarter of the ctok term
                    for j in range(4):
                        nc.vector.tensor_scalar_mul(ctokT[j][:], ctokT[j][:],
                                                    0.25)
                fl = list(fill_sched.get(h, []))
                for kt in range(8):
                    if h < NH:
                        emit_sc(h, kt)
                    if h > 0:
                        emit_o(h - 1, kt)
                    if kt % 2 == 1 and fl:
                        fl.pop(0)()
                if h > 0:
                    norm(h - 1, o_tiles.pop(h - 1))
                    if h >= 5:
                        merge(h - 5)  # pairs merge as they complete
            # last cf-side conv chunk overlaps the h7 normalization
            emit_cc(3)

            # ---- proj (s chunks stay in SBUF; rows are parity-ordered) ----
            def emit_proj(t):
                acc = sct()
                for j in range(4):
                    nc.tensor.matmul(acc[:, 0:512],
                                     pair[j][:, 128 * t:128 * (t + 1)],
                                     wproj_v[:, j, :],
                                     start=(j == 0), stop=(j == 3))
                nc.scalar.copy(s_sb[t][:], acc[:, 0:512])

            def emit_cs(oc, pc):
                acc = sct()
                for k in range(4):
                    nc.tensor.matmul(acc[:, 0:512],
                                     wcs_v[:, k, 128 * oc:128 * (oc + 1)],
                                     s_sb[4 * pc + k][:],
                                     start=(k == 0), stop=(k == 3))
                nc.vector.tensor_add(out_sb[oc][:, 512 * pc:512 * (pc + 1)],
                                     acc[:, 0:512],
                                     outcf[oc][:, 512 * pc:512 * (pc + 1)])
                if pc == 1:
                    nc.sync.dma_start(out_p[128 * oc:128 * (oc + 1), :],
                                      out_sb[oc][:])

            for t in range(4):
                emit_proj(t)
            for t in range(4, 8):
                emit_proj(t)
                emit_cs(t - 4, 0)
            for oc in range(4):
                emit_cs(oc, 1)

    nc.compile()
    _CACHE["nc"] = nc
    return nc


def _shard_inputs(content_feat, components, pos_emb, Wq, Wkv, Wproj, bproj,
                  Wconv, bconv):
    h = np.float16
    ca = np.ascontiguousarray
    posT = ca(pos_emb.reshape(S, C).T, dtype=h)
    wk = ca(Wkv[:, :C], dtype=h)
    wv = ca(Wkv[:, C:], dtype=h)
    wq = ca(Wq, dtype=h)
    wp = ca(Wproj, dtype=h)
    wcT = ca(Wconv.T, dtype=h)
    wcs = ca(wcT[:C])
    wccf_first = ca(wcT[C:])
    wccf_rest = np.zeros_like(wccf_first)
    bp4 = ca(bproj.reshape(1, C) / 4.0, dtype=h)
    bc4 = ca(bconv.reshape(1, C) / 4.0, dtype=h)
    wcsum = ca(Wconv[:, :C].sum(axis=1).reshape(1, C), dtype=h)
    in_maps = []
    for core in range(N_CORES):
        b, n = core // 4, core % 4
        in_maps.append({
            "compT": ca(components[n, b].reshape(S, C).T, dtype=h),
            "posT": posT,
            "cfT": ca(content_feat[b].reshape(S, C).T, dtype=h),
            "cfc": ca(content_feat[b].reshape(C, S), dtype=h),
            "wk": wk, "wv": wv, "wq": wq, "wproj": wp,
            "wcs": wcs,
            "wccf": wccf_first if n == 0 else wccf_rest,
            "bp4": bp4, "bc4": bc4, "wcsum": wcsum,
        })
    return in_maps


def _run(trace=False, **inputs):
    from concourse.bass_utils import run_bass_kernel_spmd

    nc = _build()
    in_maps = _shard_inputs(**inputs)
    res = run_bass_kernel_spmd(nc, in_maps, list(range(N_CORES)), trace=trace)
    outs = [res.results[i]["out_p"].astype(np.float32) for i in range(N_CORES)]
    out = np.stack([outs[0] + outs[1] + outs[2] + outs[3],
                    outs[4] + outs[5] + outs[6] + outs[7]], axis=0)
    return out.reshape(B, C, H, W).astype(np.float32), res


def kernel(**inputs):
    out, _ = _run(trace=False, **inputs)
    return out


# revision 28
# speedup vs baseline: 1.2454x; 1.0400x over previous
"""Trainium2 Bass kernel for nn_Attention_54391465836966.

Math (per batch b, component n; one (b, n) pair per core):
  ctok = content_feat[b].reshape(S,C) + pos           # raw reshape tokens
  comp_tok = components[n,b].reshape(S,C) + pos
  q = ctok @ Wq ; k,v = comp_tok @ Wkv (split)
  per head h: P' = exp(scale*q k^T - 12); o_h = (P' @ v) / rowsum(P')
  result = sum_n o_n ; s = (result + ctok) @ Wproj + bproj
  out = Wconv[:, :512] . s2d + Wconv[:, 512:] . cf2d + bconv
    (s2d = raw [C, S] view of the token-major s buffer)

All device data is fp16 (f32 PSUM accumulation); exp carries a constant
-12 bias so probabilities fit fp16 (softmax is invariant to it).  Host
passes token-channel-major transposes (comp^T, cf^T, pos^T) and Wconv^T,
so the kernel does zero PE transposes.  Query tokens are parity-permuted
(even tokens first) end-to-end: the proj output tiles then directly ARE
the raw-reshape s2d chunks the conv needs, removing the DRAM round-trip.
The (result + ctok) constant term rides the attention output as
+0.25*ctok^T per core (host sums 4 component partials per batch); the
bproj term's conv contribution is the rank-1 update colsum(Wconv_s) x
bproj/4, folded into the cf-side conv.

Schedule: the attention exp stream (ACT engine) is the longest chain, so
everything else is threaded through its gaps -- v/kT/qT setup matmuls
fill heads 0-3 and the cf-side conv chunks fill heads 4-7, keeping exp
contiguous from ~14us on.  Normalization copies o out of PSUM once
(freeing the single o bank pair), takes 1/Z via the fast DVE reciprocal
from the ones-column row, and scales into head-pair tiles consumed by
the proj/conv tail.
"""
import sys

sys.path.insert(0, "/opt/trn_rl_repo")

import numpy as np

N_CORES = 8
B, C, H, W = 2, 512, 32, 32
S = H * W  # 1024
NH, HD = 8, 64
SCALE = HD ** -0.5
EXP_BIAS = -12.0

_CACHE = {}


def _build():
    if "nc" in _CACHE:
        return _CACHE["nc"]
    from contextlib import ExitStack

    import concourse.bacc as bacc
    import concourse.mybir as mybir
    import concourse.tile as tile

    f16 = mybir.dt.float16
    f32 = mybir.dt.float32
    EXP = mybir.ActivationFunctionType.Exp

    nc = bacc.Bacc("TRN2", target_bir_lowering=False, debug=False,
                   num_devices=N_CORES)

    din = lambda n, s, dt=f16: nc.dram_tensor(n, s, dt, kind="ExternalInput").ap()
    compT_d = din("compT", [C, S])   # components[n,b] token-chan-major (host .T)
    posT_d = din("posT", [C, S])     # pos^T
    cfT_d = din("cfT", [C, S])       # content tokens^T
    cfc_d = din("cfc", [C, S])       # content real-channel-major (conv input)
    wk_d = din("wk", [C, C])         # Wkv[:, :C]
    wv_d = din("wv", [C, C])         # Wkv[:, C:]
    wq_d = din("wq", [C, C])
    wproj_d = din("wproj", [C, C])
    wcs_d = din("wcs", [C, C])       # Wconv^T rows 0:C   (s part)
    wccf_d = din("wccf", [C, C])     # Wconv^T rows C:2C  (cf part; 0 for n>0)
    bp4_d = din("bp4", [1, C])       # bproj/4
    bc4_d = din("bc4", [1, C])       # bconv/4
    wcsum_d = din("wcsum", [1, C])   # colsum of Wconv s-half (rank-1 bias term)
    out_p = nc.dram_tensor("out_p", [C, S], f16, kind="ExternalOutput").ap()

    wview = lambda d: d.rearrange("(k p) c -> p k c", p=128)
    tview = lambda t: t[:].rearrange("p (k c) -> p k c", k=4)

    with tile.TileContext(nc) as tc, ExitStack() as ctx:
        main = ctx.enter_context(tc.tile_pool(name="main", bufs=1))
        trans = ctx.enter_context(tc.tile_pool(name="trans", bufs=2))

        # ---- input DMAs (SP ring, in order of first use) ----
        compT_raw = [main.tile([128, S], f16, tag=f"cr{j}", name=f"compTr{j}")
                     for j in range(4)]
        posT = [main.tile([128, S], f16, tag=f"pos{j}", name=f"posT{j}")
                for j in range(4)]
        cfT = [main.tile([128, S], f16, tag=f"cfT{j}", name=f"cfT{j}")
               for j in range(4)]
        for j in range(4):
            nc.sync.dma_start(compT_raw[j][:], compT_d[128 * j:128 * (j + 1), :])
            nc.sync.dma_start(posT[j][:], posT_d[128 * j:128 * (j + 1), :])
            nc.sync.dma_start(cfT[j][:], cfT_d[128 * j:128 * (j + 1), :])
        wk = main.tile([128, 4 * C], f16, tag="wk")
        wq = main.tile([128, 4 * C], f16, tag="wq")
        wv = main.tile([128, 4 * C], f16, tag="wv")
        nc.sync.dma_start(tview(wk), wview(wk_d)[:, :, :])
        nc.sync.dma_start(tview(wq), wview(wq_d)[:, :, :])
        nc.sync.dma_start(tview(wv), wview(wv_d)[:, :, :])
        wproj = main.tile([128, 4 * C], f16, tag="wp")
        wcs = main.tile([128, 4 * C], f16, tag="wcs")
        wccf = main.tile([128, 4 * C], f16, tag="wccf")
        nc.sync.dma_start(tview(wproj), wview(wproj_d)[:, :, :])
        nc.sync.dma_start(tview(wcs), wview(wcs_d)[:, :, :])
        nc.sync.dma_start(tview(wccf), wview(wccf_d)[:, :, :])
        cfc = [main.tile([128, S], f16, tag=f"cfc{j}", name=f"cfc{j}")
               for j in range(4)]
        for j in range(4):
            nc.sync.dma_start(cfc[j][:], cfc_d[128 * j:128 * (j + 1), :])
        bp4 = main.tile([1, C], f16, tag="bp4")
        bc4 = main.tile([1, C], f16, tag="bc4")
        wcsum = main.tile([1, C], f16, tag="wcsum")
        nc.sync.dma_start(bp4[:], bp4_d[:])
        nc.sync.dma_start(bc4[:], bc4_d[:])
        nc.sync.dma_start(wcsum[:], wcsum_d[:])

        wk_v, wq_v, wv_v = tview(wk), tview(wq), tview(wv)
        wproj_v, wcs_v, wccf_v = tview(wproj), tview(wcs), tview(wccf)

        ones = main.tile([128, S], f16, tag="ones")
        nc.gpsimd.memset(ones[:], 1.0)
        ebias = main.tile([128, 1], f32, tag="ebias")
        nc.gpsimd.memset(ebias[:], EXP_BIAS)
        # pull the ACT function-table load off the critical path
        actwarm = main.tile([1, 1], f16, tag="actwarm")
        nc.scalar.activation(actwarm[0:1, 0:1], ebias[0:1, 0:1], EXP,
                             bias=ebias[0:1, 0:1], scale=SCALE)

        # ---- token adds ----
        tok = [main.tile([128, S], f16, tag=f"tok{j}", name=f"tok{j}")
               for j in range(4)]
        for j in range(4):
            nc.vector.tensor_add(tok[j][:], compT_raw[j][:], posT[j][:])
        # content tokens: parity-permuted columns (even tokens then odd)
        ctokT = [main.tile([128, S], f16, tag=f"ctokT{j}", name=f"ctokT{j}")
                 for j in range(4)]
        ev = lambda ap, par: ap.rearrange("p (a two) -> p a two", two=2)[
            :, :, par:par + 1]
        for j in range(4):
            for par in range(2):
                dst = ctokT[j][:, 512 * par:512 * (par + 1)].rearrange(
                    "p (a o) -> p a o", o=1)
                nc.vector.tensor_add(dst, ev(cfT[j][:], par), ev(posT[j][:], par))

        kT = [main.tile([128, S], f16, tag=f"kT{j}", name=f"kT{j}")
              for j in range(4)]
        qT = [main.tile([128, S], f16, tag=f"qT{j}", name=f"qT{j}")
              for j in range(4)]
        v_sb = [main.tile([128, 8 * 65], f16, tag=f"v{t}", name=f"v{t}")
                for t in range(8)]
        vv = lambda t: v_sb[t][:].rearrange("p (h e) -> p h e", h=8)
        for t in range(8):
            # ones column LAST (e=64): Z lands on o_ps partition 64 (aligned)
            nc.gpsimd.tensor_copy(vv(t)[:, :, 64:65],
                                  ones[:, 0:8].rearrange("p (h w) -> p h w", w=1))
        pair = [main.tile([128, S], f16, tag=f"pair{j}", name=f"pair{j}")
                for j in range(4)]
        outcf = [main.tile([128, S], f32, tag=f"ocf{oc}", name=f"ocf{oc}")
                 for oc in range(4)]
        s_sb = [main.tile([128, C], f16, tag=f"s{t}", name=f"s{t}")
                for t in range(8)]
        out_sb = [main.tile([128, S], f16, tag=f"ob{oc}", name=f"ob{oc}")
                  for oc in range(4)]

        with tc.tile_pool(name="ps", bufs=1, space="PSUM") as ps:
            _n = [0]

            def pst(tag, shape, bufs):
                _n[0] += 1
                return ps.tile(shape, f32, tag=tag, bufs=bufs,
                               name=f"{tag}{_n[0]}")

            sct = lambda: pst("sc", [128, S], 2)

            # ramp the PE p-state during the DMA shadow (outputs unused)
            warm_ps = sct()
            for _ in range(28):
                nc.tensor.matmul(warm_ps[:, 0:512], ones[:, 0:128],
                                 ones[:, 0:512], start=True, stop=True)

            # ---- filler emitters (run inside attention's ACT-bound gaps) --
            def emit_kq(dst, wsrc, act, j, tck, on_act=False):
                acc = sct()
                for k in range(4):
                    nc.tensor.matmul(acc[:, 0:512],
                                     wsrc[:, k, 128 * j:128 * (j + 1)],
                                     act[k][:, 512 * tck:512 * (tck + 1)],
                                     start=(k == 0), stop=(k == 3))
                if on_act:
                    # lead-in only: ACT is idle before the exp stream starts
                    nc.scalar.copy(dst[j][:, 512 * tck:512 * (tck + 1)],
                                   acc[:, 0:512])
                else:
                    nc.vector.tensor_copy(dst[j][:, 512 * tck:512 * (tck + 1)],
                                          acc[:, 0:512])

            def emit_v(t):
                acc = sct()
                for k in range(4):
                    nc.tensor.matmul(acc[:, 0:512],
                                     tok[k][:, 128 * t:128 * (t + 1)],
                                     wv_v[:, k, :],
                                     start=(k == 0), stop=(k == 3))
                nc.vector.tensor_copy(
                    vv(t)[:, :, 0:64],
                    acc[:, 0:512].rearrange("p (h d) -> p h d", h=8))

            def emit_cc(oc):
                # cf-half conv + quartered bconv + rank-1 bproj term
                acc = sct()
                for pc in range(2):
                    half = acc[:, 512 * pc:512 * (pc + 1)]
                    nc.tensor.matmul(half, bc4[0:1, 128 * oc:128 * (oc + 1)],
                                     ones[0:1, 0:512], start=True, stop=False)
                    nc.tensor.matmul(half, wcsum[0:1, 128 * oc:128 * (oc + 1)],
                                     bp4[0:1, :], start=False, stop=False)
                    for k in range(4):
                        nc.tensor.matmul(half,
                                         wccf_v[:, k, 128 * oc:128 * (oc + 1)],
                                         cfc[k][:, 512 * pc:512 * (pc + 1)],
                                         start=False, stop=(k == 3))
                nc.vector.tensor_copy(outcf[oc][:], acc[:])

            # ---- lead-in: kT0 / qT0 (copies on the still-idle ACT) ----
            emit_kq(kT, wk_v, tok, 0, 0, on_act=True)
            emit_kq(kT, wk_v, tok, 0, 1, on_act=True)
            emit_kq(qT, wq_v, ctokT, 0, 0, on_act=True)
            emit_kq(qT, wq_v, ctokT, 0, 1, on_act=True)

            # ---- attention, software-pipelined one head deep ----
            # Block h emits head h's scores+exp (the ACT critical stream)
            # interleaved per-kt with head h-1's o matmuls (whose exps
            # finished a full block ago -> PE never waits on ACT) plus one
            # filler group; a drain block finishes head 7.
            def norm(h, o_ps):
                jq, row = h // 2, 64 * (h % 2)
                zscr = trans.tile([1, S], f32, tag="zscr", bufs=2, name=f"zs{h}")
                zinv = trans.tile([1, S], f32, tag="zinv", bufs=2, name=f"zi{h}")
                zb = trans.tile([64, S], f32, tag="zb", bufs=2, name=f"zb{h}")
                nc.vector.tensor_copy(zscr[0:1, :], o_ps[64:65, :])
                nc.vector.reciprocal_approx_fast(zinv[0:1, :], zscr[0:1, :])
                nc.gpsimd.partition_broadcast(zb[0:64, :], zinv[0:1, :])
                nc.vector.tensor_mul(pair[jq][row:row + 64, :],
                                     o_ps[0:64, :], zb[0:64, :])

            merge = lambda j: nc.vector.tensor_add(pair[j][:], pair[j][:],
                                                   ctokT[j][:])
            fill_sched = {
                0: [lambda t=t: emit_v(t) for t in range(8)],
                1: [lambda tc=tc: emit_kq(kT, wk_v, tok, 1, tc) for tc in (0, 1)]
                 + [lambda tc=tc: emit_kq(qT, wq_v, ctokT, 1, tc) for tc in (0, 1)],
                2: [lambda tc=tc: emit_kq(kT, wk_v, tok, 2, tc) for tc in (0, 1)]
                 + [lambda tc=tc: emit_kq(qT, wq_v, ctokT, 2, tc) for tc in (0, 1)],
                3: [lambda tc=tc: emit_kq(kT, wk_v, tok, 3, tc) for tc in (0, 1)]
                 + [lambda tc=tc: emit_kq(qT, wq_v, ctokT, 3, tc) for tc in (0, 1)],
                4: [lambda: emit_cc(0)],
                5: [lambda: emit_cc(1)],
                6: [lambda: emit_cc(2)],
            }
            pts = {}
            o_tiles = {}

            def emit_sc(h, kt):
                jq, row = h // 2, 64 * (h % 2)
                sc = sct()
                for qc in range(2):
                    nc.tensor.matmul(
                        sc[:, 512 * qc:512 * (qc + 1)],
                        kT[jq][row:row + 64, 128 * kt:128 * (kt + 1)],
                        qT[jq][row:row + 64, 512 * qc:512 * (qc + 1)],
                        start=True, stop=True)
                pt = trans.tile([128, S], f16, tag="pt", bufs=12,
                                name=f"pt{h}_{kt}")
                nc.scalar.activation(pt[:], sc[:], EXP, bias=ebias[:, 0:1],
                                     scale=SCALE)
                pts[(h, kt)] = pt

            def emit_o(h, kt):
                if kt == 0:
                    o_tiles[h] = pst("o", [65, S], 2)
                o_ps = o_tiles[h]
                for qc in range(2):
                    nc.tensor.matmul(
                        o_ps[:, 512 * qc:512 * (qc + 1)],
                        vv(kt)[:, h, :],
                        pts[(h, kt)][:, 512 * qc:512 * (qc + 1)],
                        start=(kt == 0), stop=(kt == 7))
                if kt == 7:
                    del pts[(h, kt)]

            for h in range(NH + 1):
                if h == 4:
                    # all qT consumers of ctokT are done: scale in place; each
                    # component core contributes a quarter of the ctok term
                    for j in range(4):
                        nc.vector.tensor_scalar_mul(ctokT[j][:], ctokT[j][:],
                                                    0.25)
                fl = list(fill_sched.get(h, []))
                for kt in range(8):
                    if h < NH:
                        emit_sc(h, kt)
                    if h > 0:
                        emit_o(h - 1, kt)
                    if fl and (h == 0 or kt % 2 == 1):
                        fl.pop(0)()
                if h > 0:
                    norm(h - 1, o_tiles.pop(h - 1))
                    if h >= 5:
                        merge(h - 5)  # pairs merge as they complete
            # last cf-side conv chunk overlaps the h7 normalization
            emit_cc(3)

            # ---- proj (s chunks stay in SBUF; rows are parity-ordered) ----
            def emit_proj(t):
                acc = sct()
                for j in range(4):
                    nc.tensor.matmul(acc[:, 0:512],
                                     pair[j][:, 128 * t:128 * (t + 1)],
                                     wproj_v[:, j, :],
                                     start=(j == 0), stop=(j == 3))
                nc.scalar.copy(s_sb[t][:], acc[:, 0:512])

            def emit_cs(oc, pc):
                acc = sct()
                for k in range(4):
                    nc.tensor.matmul(acc[:, 0:512],
                                     wcs_v[:, k, 128 * oc:128 * (oc + 1)],
                                     s_sb[4 * pc + k][:],
                                     start=(k == 0), stop=(k == 3))
                nc.vector.tensor_add(out_sb[oc][:, 512 * pc:512 * (pc + 1)],
                                     acc[:, 0:512],
                                     outcf[oc][:, 512 * pc:512 * (pc + 1)])
                if pc == 1:
                    nc.sync.dma_start(out_p[128 * oc:128 * (oc + 1), :],
                                      out_sb[oc][:])

            for t in range(4):
                emit_proj(t)
            for t in range(4, 8):
                emit_proj(t)
                emit_cs(t - 4, 0)
            for oc in range(4):
                emit_cs(oc, 1)

    nc.compile()
    _CACHE["nc"] = nc
    return nc


def _shard_inputs(content_feat, components, pos_emb, Wq, Wkv, Wproj, bproj,
                  Wconv, bconv):
    h = np.float16
    ca = np.ascontiguousarray
    posT = ca(pos_emb.reshape(S, C).T, dtype=h)
    wk = ca(Wkv[:, :C], dtype=h)
    wv = ca(Wkv[:, C:], dtype=h)
    wq = ca(Wq, dtype=h)
    wp = ca(Wproj, dtype=h)
    wcT = ca(Wconv.T, dtype=h)
    wcs = ca(wcT[:C])
    wccf_first = ca(wcT[C:])
    wccf_rest = np.zeros_like(wccf_first)
    bp4 = ca(bproj.reshape(1, C) / 4.0, dtype=h)
    bc4 = ca(bconv.reshape(1, C) / 4.0, dtype=h)
    wcsum = ca(Wconv[:, :C].sum(axis=1).reshape(1, C), dtype=h)
    in_maps = []
    for core in range(N_CORES):
        b, n = core // 4, core % 4
        in_maps.append({
            "compT": ca(components[n, b].reshape(S, C).T, dtype=h),
            "posT": posT,
            "cfT": ca(content_feat[b].reshape(S, C).T, dtype=h),
            "cfc": ca(content_feat[b].reshape(C, S), dtype=h),
            "wk": wk, "wv": wv, "wq": wq, "wproj": wp,
            "wcs": wcs,
            "wccf": wccf_first if n == 0 else wccf_rest,
            "bp4": bp4, "bc4": bc4, "wcsum": wcsum,
        })
    return in_maps


def _run(trace=False, **inputs):
    from concourse.bass_utils import run_bass_kernel_spmd

    nc = _build()
    in_maps = _shard_inputs(**inputs)
    res = run_bass_kernel_spmd(nc, in_maps, list(range(N_CORES)), trace=trace)
    outs = [res.results[i]["out_p"].astype(np.float32) for i in range(N_CORES)]
    out = np.stack([outs[0] + outs[1] + outs[2] + outs[3],
                    outs[4] + outs[5] + outs[6] + outs[7]], axis=0)
    return out.reshape(B, C, H, W).astype(np.float32), res


def kernel(**inputs):
    out, _ = _run(trace=False, **inputs)
    return out


# revision 29
# speedup vs baseline: 1.2524x; 1.0056x over previous
"""Trainium2 Bass kernel for nn_Attention_54391465836966.

Math (per batch b, component n; one (b, n) pair per core):
  ctok = content_feat[b].reshape(S,C) + pos           # raw reshape tokens
  comp_tok = components[n,b].reshape(S,C) + pos
  q = ctok @ Wq ; k,v = comp_tok @ Wkv (split)
  per head h: P' = exp(scale*q k^T - 12); o_h = (P' @ v) / rowsum(P')
  result = sum_n o_n ; s = (result + ctok) @ Wproj + bproj
  out = Wconv[:, :512] . s2d + Wconv[:, 512:] . cf2d + bconv
    (s2d = raw [C, S] view of the token-major s buffer)

All device data is fp16 (f32 PSUM accumulation); exp carries a constant
-12 bias so probabilities fit fp16 (softmax is invariant to it).  Host
passes token-channel-major transposes (comp^T, cf^T, pos^T) and Wconv^T,
so the kernel does zero PE transposes.  Query tokens are parity-permuted
(even tokens first) end-to-end: the proj output tiles then directly ARE
the raw-reshape s2d chunks the conv needs, removing the DRAM round-trip.
The (result + ctok) constant term rides the attention output as
+0.25*ctok^T per core (host sums 4 component partials per batch); the
bproj term's conv contribution is the rank-1 update colsum(Wconv_s) x
bproj/4, folded into the cf-side conv.

Schedule: the attention exp stream (ACT engine) is the longest chain, so
everything else is threaded through its gaps -- v/kT/qT setup matmuls
fill heads 0-3 and the cf-side conv chunks fill heads 4-7, keeping exp
contiguous from ~14us on.  Normalization copies o out of PSUM once
(freeing the single o bank pair), takes 1/Z via the fast DVE reciprocal
from the ones-column row, and scales into head-pair tiles consumed by
the proj/conv tail.
"""
import sys

sys.path.insert(0, "/opt/trn_rl_repo")

import numpy as np

N_CORES = 8
B, C, H, W = 2, 512, 32, 32
S = H * W  # 1024
NH, HD = 8, 64
SCALE = HD ** -0.5
EXP_BIAS = -12.0

_CACHE = {}


def _build():
    if "nc" in _CACHE:
        return _CACHE["nc"]
    from contextlib import ExitStack

    import concourse.bacc as bacc
    import concourse.mybir as mybir
    import concourse.tile as tile

    f16 = mybir.dt.float16
    f32 = mybir.dt.float32
    EXP = mybir.ActivationFunctionType.Exp

    nc = bacc.Bacc("TRN2", target_bir_lowering=False, debug=False,
                   num_devices=N_CORES)

    din = lambda n, s, dt=f16: nc.dram_tensor(n, s, dt, kind="ExternalInput").ap()
    compT_d = din("compT", [C, S])   # components[n,b] token-chan-major (host .T)
    posT_d = din("posT", [C, S])     # pos^T
    cfT_d = din("cfT", [C, S])       # content tokens^T
    cfc_d = din("cfc", [C, S])       # content real-channel-major (conv input)
    wk_d = din("wk", [C, C])         # Wkv[:, :C]
    wv_d = din("wv", [C, C])         # Wkv[:, C:]
    wq_d = din("wq", [C, C])
    wproj_d = din("wproj", [C, C])
    wcs_d = din("wcs", [C, C])       # Wconv^T rows 0:C   (s part)
    wccf_d = din("wccf", [C, C])     # Wconv^T rows C:2C  (cf part; 0 for n>0)
    bp4_d = din("bp4", [1, C])       # bproj/4
    bc4_d = din("bc4", [1, C])       # bconv/4
    wcsum_d = din("wcsum", [1, C])   # colsum of Wconv s-half (rank-1 bias term)
    out_p = nc.dram_tensor("out_p", [C, S], f16, kind="ExternalOutput").ap()

    wview = lambda d: d.rearrange("(k p) c -> p k c", p=128)
    tview = lambda t: t[:].rearrange("p (k c) -> p k c", k=4)

    with tile.TileContext(nc) as tc, ExitStack() as ctx:
        main = ctx.enter_context(tc.tile_pool(name="main", bufs=1))
        trans = ctx.enter_context(tc.tile_pool(name="trans", bufs=2))

        # ---- input DMAs (SP ring, in order of first use) ----
        compT_raw = [main.tile([128, S], f16, tag=f"cr{j}", name=f"compTr{j}")
                     for j in range(4)]
        posT = [main.tile([128, S], f16, tag=f"pos{j}", name=f"posT{j}")
                for j in range(4)]
        cfT = [main.tile([128, S], f16, tag=f"cfT{j}", name=f"cfT{j}")
               for j in range(4)]
        for j in range(4):
            nc.sync.dma_start(compT_raw[j][:], compT_d[128 * j:128 * (j + 1), :])
            nc.sync.dma_start(posT[j][:], posT_d[128 * j:128 * (j + 1), :])
            nc.sync.dma_start(cfT[j][:], cfT_d[128 * j:128 * (j + 1), :])
        wk = main.tile([128, 4 * C], f16, tag="wk")
        wq = main.tile([128, 4 * C], f16, tag="wq")
        wv = main.tile([128, 4 * C], f16, tag="wv")
        nc.sync.dma_start(tview(wk), wview(wk_d)[:, :, :])
        nc.sync.dma_start(tview(wq), wview(wq_d)[:, :, :])
        nc.sync.dma_start(tview(wv), wview(wv_d)[:, :, :])
        wproj = main.tile([128, 4 * C], f16, tag="wp")
        wcs = main.tile([128, 4 * C], f16, tag="wcs")
        wccf = main.tile([128, 4 * C], f16, tag="wccf")
        nc.sync.dma_start(tview(wproj), wview(wproj_d)[:, :, :])
        nc.sync.dma_start(tview(wcs), wview(wcs_d)[:, :, :])
        nc.sync.dma_start(tview(wccf), wview(wccf_d)[:, :, :])
        cfc = [main.tile([128, S], f16, tag=f"cfc{j}", name=f"cfc{j}")
               for j in range(4)]
        for j in range(4):
            nc.sync.dma_start(cfc[j][:], cfc_d[128 * j:128 * (j + 1), :])
        bp4 = main.tile([1, C], f16, tag="bp4")
        bc4 = main.tile([1, C], f16, tag="bc4")
        wcsum = main.tile([1, C], f16, tag="wcsum")
        nc.sync.dma_start(bp4[:], bp4_d[:])
        nc.sync.dma_start(bc4[:], bc4_d[:])
        nc.sync.dma_start(wcsum[:], wcsum_d[:])

        wk_v, wq_v, wv_v = tview(wk), tview(wq), tview(wv)
        wproj_v, wcs_v, wccf_v = tview(wproj), tview(wcs), tview(wccf)

        ones = main.tile([128, S], f16, tag="ones")
        nc.gpsimd.memset(ones[:], 1.0)
        ebias = main.tile([128, 1], f32, tag="ebias")
        nc.gpsimd.memset(ebias[:], EXP_BIAS)
        # pull the ACT function-table load off the critical path
        actwarm = main.tile([1, 1], f16, tag="actwarm")
        nc.scalar.activation(actwarm[0:1, 0:1], ebias[0:1, 0:1], EXP,
                             bias=ebias[0:1, 0:1], scale=SCALE)

        # ---- token adds ----
        tok = [main.tile([128, S], f16, tag=f"tok{j}", name=f"tok{j}")
               for j in range(4)]
        for j in range(4):
            nc.vector.tensor_add(tok[j][:], compT_raw[j][:], posT[j][:])
        # content tokens: parity-permuted columns (even tokens then odd)
        ctokT = [main.tile([128, S], f16, tag=f"ctokT{j}", name=f"ctokT{j}")
                 for j in range(4)]
        ev = lambda ap, par: ap.rearrange("p (a two) -> p a two", two=2)[
            :, :, par:par + 1]
        for j in range(4):
            for par in range(2):
                dst = ctokT[j][:, 512 * par:512 * (par + 1)].rearrange(
                    "p (a o) -> p a o", o=1)
                nc.vector.tensor_add(dst, ev(cfT[j][:], par), ev(posT[j][:], par))

        kT = [main.tile([128, S], f16, tag=f"kT{j}", name=f"kT{j}")
              for j in range(4)]
        qT = [main.tile([128, S], f16, tag=f"qT{j}", name=f"qT{j}")
              for j in range(4)]
        v_sb = [main.tile([128, 8 * 65], f16, tag=f"v{t}", name=f"v{t}")
                for t in range(8)]
        vv = lambda t: v_sb[t][:].rearrange("p (h e) -> p h e", h=8)
        for t in range(8):
            # ones column LAST (e=64): Z lands on o_ps partition 64 (aligned)
            nc.gpsimd.tensor_copy(vv(t)[:, :, 64:65],
                                  ones[:, 0:8].rearrange("p (h w) -> p h w", w=1))
        pair = [main.tile([128, S], f16, tag=f"pair{j}", name=f"pair{j}")
                for j in range(4)]
        outcf = [main.tile([128, S], f32, tag=f"ocf{oc}", name=f"ocf{oc}")
                 for oc in range(4)]
        s_sb = [main.tile([128, C], f16, tag=f"s{t}", name=f"s{t}")
                for t in range(8)]
        out_sb = [main.tile([128, S], f16, tag=f"ob{oc}", name=f"ob{oc}")
                  for oc in range(4)]

        with tc.tile_pool(name="ps", bufs=1, space="PSUM") as ps:
            _n = [0]

            def pst(tag, shape, bufs):
                _n[0] += 1
                return ps.tile(shape, f32, tag=tag, bufs=bufs,
                               name=f"{tag}{_n[0]}")

            sct = lambda: pst("sc", [128, 512], 4)

            # ramp the PE p-state during the DMA shadow (outputs unused)
            warm_ps = sct()
            for _ in range(28):
                nc.tensor.matmul(warm_ps[:], ones[:, 0:128],
                                 ones[:, 0:512], start=True, stop=True)

            # ---- filler emitters (run inside attention's ACT-bound gaps) --
            def emit_kq(dst, wsrc, act, j, tck, on_act=False):
                acc = sct()
                for k in range(4):
                    nc.tensor.matmul(acc[:],
                                     wsrc[:, k, 128 * j:128 * (j + 1)],
                                     act[k][:, 512 * tck:512 * (tck + 1)],
                                     start=(k == 0), stop=(k == 3))
                if on_act:
                    # lead-in only: ACT is idle before the exp stream starts
                    nc.scalar.copy(dst[j][:, 512 * tck:512 * (tck + 1)],
                                   acc[:])
                else:
                    nc.vector.tensor_copy(dst[j][:, 512 * tck:512 * (tck + 1)],
                                          acc[:])

            def emit_v(t):
                acc = sct()
                for k in range(4):
                    nc.tensor.matmul(acc[:],
                                     tok[k][:, 128 * t:128 * (t + 1)],
                                     wv_v[:, k, :],
                                     start=(k == 0), stop=(k == 3))
                nc.vector.tensor_copy(
                    vv(t)[:, :, 0:64],
                    acc[:].rearrange("p (h d) -> p h d", h=8))

            def emit_cc(oc):
                # cf-half conv + quartered bconv + rank-1 bproj term
                for pc in range(2):
                    half = sct()
                    nc.tensor.matmul(half[:], bc4[0:1, 128 * oc:128 * (oc + 1)],
                                     ones[0:1, 0:512], start=True, stop=False)
                    nc.tensor.matmul(half[:], wcsum[0:1, 128 * oc:128 * (oc + 1)],
                                     bp4[0:1, :], start=False, stop=False)
                    for k in range(4):
                        nc.tensor.matmul(half[:],
                                         wccf_v[:, k, 128 * oc:128 * (oc + 1)],
                                         cfc[k][:, 512 * pc:512 * (pc + 1)],
                                         start=False, stop=(k == 3))
                    nc.vector.tensor_copy(outcf[oc][:, 512 * pc:512 * (pc + 1)],
                                          half[:])

            # ---- lead-in: kT0 / qT0 (copies on the still-idle ACT) ----
            emit_kq(kT, wk_v, tok, 0, 0, on_act=True)
            emit_kq(kT, wk_v, tok, 0, 1, on_act=True)
            emit_kq(qT, wq_v, ctokT, 0, 0, on_act=True)
            emit_kq(qT, wq_v, ctokT, 0, 1, on_act=True)

            # ---- attention, software-pipelined one head deep ----
            # Block h emits head h's scores+exp (the ACT critical stream)
            # interleaved per-kt with head h-1's o matmuls (whose exps
            # finished a full block ago -> PE never waits on ACT) plus one
            # filler group; a drain block finishes head 7.
            def norm(h, o_ps):
                jq, row = h // 2, 64 * (h % 2)
                zscr = trans.tile([1, S], f32, tag="zscr", bufs=2, name=f"zs{h}")
                zinv = trans.tile([1, S], f32, tag="zinv", bufs=2, name=f"zi{h}")
                zb = trans.tile([64, S], f32, tag="zb", bufs=2, name=f"zb{h}")
                nc.vector.tensor_copy(zscr[0:1, :], o_ps[64:65, :])
                nc.vector.reciprocal_approx_fast(zinv[0:1, :], zscr[0:1, :])
                nc.gpsimd.partition_broadcast(zb[0:64, :], zinv[0:1, :])
                nc.vector.tensor_mul(pair[jq][row:row + 64, :],
                                     o_ps[0:64, :], zb[0:64, :])

            merge = lambda j: nc.vector.tensor_add(pair[j][:], pair[j][:],
                                                   ctokT[j][:])
            fill_sched = {
                0: [lambda t=t: emit_v(t) for t in range(8)],
                1: [lambda tc=tc: emit_kq(kT, wk_v, tok, 1, tc) for tc in (0, 1)]
                 + [lambda tc=tc: emit_kq(qT, wq_v, ctokT, 1, tc) for tc in (0, 1)],
                2: [lambda tc=tc: emit_kq(kT, wk_v, tok, 2, tc) for tc in (0, 1)]
                 + [lambda tc=tc: emit_kq(qT, wq_v, ctokT, 2, tc) for tc in (0, 1)],
                3: [lambda tc=tc: emit_kq(kT, wk_v, tok, 3, tc) for tc in (0, 1)]
                 + [lambda tc=tc: emit_kq(qT, wq_v, ctokT, 3, tc) for tc in (0, 1)],
                4: [lambda: emit_cc(0)],
                5: [lambda: emit_cc(1)],
                6: [lambda: emit_cc(2)],
            }
            pts = {}
            o_tiles = {}

            def emit_sc(h, kt):
                jq, row = h // 2, 64 * (h % 2)
                pt = trans.tile([128, S], f16, tag="pt", bufs=12,
                                name=f"pt{h}_{kt}")
                for qc in range(2):
                    sc = sct()
                    nc.tensor.matmul(
                        sc[:],
                        kT[jq][row:row + 64, 128 * kt:128 * (kt + 1)],
                        qT[jq][row:row + 64, 512 * qc:512 * (qc + 1)],
                        start=True, stop=True)
                    nc.scalar.activation(pt[:, 512 * qc:512 * (qc + 1)], sc[:],
                                         EXP, bias=ebias[:, 0:1], scale=SCALE)
                pts[(h, kt)] = pt

            def emit_o(h, kt):
                if kt == 0:
                    o_tiles[h] = pst("o", [65, S], 2)
                o_ps = o_tiles[h]
                for qc in range(2):
                    nc.tensor.matmul(
                        o_ps[:, 512 * qc:512 * (qc + 1)],
                        vv(kt)[:, h, :],
                        pts[(h, kt)][:, 512 * qc:512 * (qc + 1)],
                        start=(kt == 0), stop=(kt == 7))
                if kt == 7:
                    del pts[(h, kt)]

            for h in range(NH + 1):
                if h == 4:
                    # all qT consumers of ctokT are done: scale in place; each
                    # component core contributes a quarter of the ctok term
                    for j in range(4):
                        nc.vector.tensor_scalar_mul(ctokT[j][:], ctokT[j][:],
                                                    0.25)
                fl = list(fill_sched.get(h, []))
                for kt in range(8):
                    if h < NH:
                        emit_sc(h, kt)
                    if h > 0:
                        emit_o(h - 1, kt)
                    if fl and (h == 0 or kt % 2 == 1):
                        fl.pop(0)()
                if h > 0:
                    norm(h - 1, o_tiles.pop(h - 1))
                    if h >= 5:
                        merge(h - 5)  # pairs merge as they complete
            # last cf-side conv chunk overlaps the h7 normalization
            emit_cc(3)

            # ---- proj (s chunks stay in SBUF; rows are parity-ordered) ----
            def emit_proj(t):
                acc = sct()
                for j in range(4):
                    nc.tensor.matmul(acc[:],
                                     pair[j][:, 128 * t:128 * (t + 1)],
                                     wproj_v[:, j, :],
                                     start=(j == 0), stop=(j == 3))
                nc.scalar.copy(s_sb[t][:], acc[:])

            def emit_cs(oc, pc):
                acc = sct()
                for k in range(4):
                    nc.tensor.matmul(acc[:],
                                     wcs_v[:, k, 128 * oc:128 * (oc + 1)],
                                     s_sb[4 * pc + k][:],
                                     start=(k == 0), stop=(k == 3))
                nc.vector.tensor_add(out_sb[oc][:, 512 * pc:512 * (pc + 1)],
                                     acc[:],
                                     outcf[oc][:, 512 * pc:512 * (pc + 1)])
                if pc == 1:
                    nc.sync.dma_start(out_p[128 * oc:128 * (oc + 1), :],
                                      out_sb[oc][:])

            for t in range(4):
                emit_proj(t)
            for t in range(4, 8):
                emit_proj(t)
                emit_cs(t - 4, 0)
            for oc in range(4):
                emit_cs(oc, 1)

    nc.compile()
    _CACHE["nc"] = nc
    return nc


def _shard_inputs(content_feat, components, pos_emb, Wq, Wkv, Wproj, bproj,
                  Wconv, bconv):
    h = np.float16
    ca = np.ascontiguousarray
    posT = ca(pos_emb.reshape(S, C).T, dtype=h)
    wk = ca(Wkv[:, :C], dtype=h)
    wv = ca(Wkv[:, C:], dtype=h)
    wq = ca(Wq, dtype=h)
    wp = ca(Wproj, dtype=h)
    wcT = ca(Wconv.T, dtype=h)
    wcs = ca(wcT[:C])
    wccf_first = ca(wcT[C:])
    wccf_rest = np.zeros_like(wccf_first)
    bp4 = ca(bproj.reshape(1, C) / 4.0, dtype=h)
    bc4 = ca(bconv.reshape(1, C) / 4.0, dtype=h)
    wcsum = ca(Wconv[:, :C].sum(axis=1).reshape(1, C), dtype=h)
    in_maps = []
    for core in range(N_CORES):
        b, n = core // 4, core % 4
        in_maps.append({
            "compT": ca(components[n, b].reshape(S, C).T, dtype=h),
            "posT": posT,
            "cfT": ca(content_feat[b].reshape(S, C).T, dtype=h),
            "cfc": ca(content_feat[b].reshape(C, S), dtype=h),
            "wk": wk, "wv": wv, "wq": wq, "wproj": wp,
            "wcs": wcs,
            "wccf": wccf_first if n == 0 else wccf_rest,
            "bp4": bp4, "bc4": bc4, "wcsum": wcsum,
        })
    return in_maps


def _run(trace=False, **inputs):
    from concourse.bass_utils import run_bass_kernel_spmd

    nc = _build()
    in_maps = _shard_inputs(**inputs)
    res = run_bass_kernel_spmd(nc, in_maps, list(range(N_CORES)), trace=trace)
    outs = [res.results[i]["out_p"].astype(np.float32) for i in range(N_CORES)]
    out = np.stack([outs[0] + outs[1] + outs[2] + outs[3],
                    outs[4] + outs[5] + outs[6] + outs[7]], axis=0)
    return out.reshape(B, C, H, W).astype(np.float32), res


def kernel(**inputs):
    out, _ = _run(trace=False, **inputs)
    return out
